# revision 13
# baseline (speedup 1.0000x reference)
"""ARMA GNN (2-layer, K=2 stacks) distributed Bass kernel for 8 TRN2 NeuronCores.

v3: v2 + GPSIMD offload — 8192-edge gather calls (8x fewer SWDGE ucode
launches), completion waits + buffer re-mark moved to the idle Scalar
engine, and DVE selector builds batched 8 chunks per instruction via
stride-0 broadcast APs.

v2: dst-sorted edge streams + PE selector-matmul segment reduction.
 - Nodes sharded 12500/core; edges partitioned by destination core.
 - Layer math refactored so message passing happens at small feature dims
   (32 cols L1, 16 cols L2), with the stack projections applied before (L1)
   or after (L2) aggregation.
 - The all-gathered per-node table is stored bf16, one node per 256B row.
 - Edge phase per layer: SWDGE dma_gather fetches src rows in dst-sorted
   order (8192 edges/call); for every 128-edge chunk the vector engine
   builds a one-hot selector S^T[j, i] = (dloc[j] == i) and the tensor
   engine accumulates S^T^T @ msg into the dst tile's PSUM bank. No
   scatter descriptors, no conflict packing; f32 PSUM accumulation.
 - agg lives in SBUF; epilogues identical in spirit to v1.
"""
import sys
import time

sys.path.insert(0, "/opt/trn_rl_repo")

import numpy as np
import ml_dtypes

import concourse.bass as bass
import concourse.bacc as bacc
import concourse.mybir as mybir
from concourse.tile import TileContext
from concourse.masks import make_identity
from concourse.library_config import mlp as mlp_lib

BF16 = ml_dtypes.bfloat16

N = 100000
E = 3200000
NC = 8
S = 12500            # nodes per core
NT = 98              # node tiles per core
SP = NT * 128        # 12544 padded nodes per core
QR = 2 * SP          # table rows per quarter (2 core shards)
TBL = NC * SP        # all-gathered table rows
BLK = int(__import__("os").environ.get("BLK", "2048"))  # edges per gather call
CHK = 128            # edges per selector chunk
SELB = 8             # selector chunks built per DVE instruction
NBUF = 3             # rotating msg buffers
NS = 8               # rotating gather-dma semaphores
FIN, HID, CLS, K = 512, 16, 40, 2

_cache = {}


def _wrap16(idx):
    """[n] int -> [128, n//16] int16: pos i at [i%16, i//16], replicated x8."""
    n = idx.shape[0]
    w = idx.astype(np.int16).reshape(n // 16, 16).T
    return np.ascontiguousarray(np.tile(w, (8, 1)))


def _preprocess(x, edge_index, iw1, rw1, b1, iw2, rw2, b2):
    src = edge_index[0].astype(np.int64)
    dst = edge_index[1].astype(np.int64)
    deg = np.bincount(dst, minlength=N).astype(np.float32)
    dinv = np.where(deg > 0, 1.0 / np.sqrt(deg), 0.0).astype(np.float32)

    core = dst // S
    q = src // (2 * S)                      # source quarter
    trow = (src // S) * SP + (src % S) - QR * q   # quarter-relative table row
    dl = dst - core * S                     # local dst row [0,12500)
    tile = dl // 128
    dloc = dl % 128

    # global (core-uniform) chunk schedule: chunks per (q, tile)
    cnt = np.zeros((NC, 4, NT), np.int64)
    np.add.at(cnt, (core, q, tile), 1)
    kqt = np.ceil(cnt.max(axis=0) / CHK).astype(np.int64)      # [4, NT]
    kqt = np.maximum(kqt, 3)   # >=3 also guards the ds-wait/ceil4 deadlock
    nchunks_q = kqt.sum(axis=1)                                # per quarter
    # call layout per quarter: calls of <=64 chunks
    calls = []                       # list of (q, chunk_off_in_stream, nchunks)
    chunk_base_q = np.zeros(4, np.int64)
    off = 0
    for qq in range(4):
        chunk_base_q[qq] = off
        rem = int(nchunks_q[qq])
        pos = off
        while rem > 0:
            take = min(BLK // CHK, rem)
            calls.append((qq, pos, take))
            pos += take
            rem -= take
        off += int(nchunks_q[qq])
    nchunks = int(off)
    strm = nchunks * CHK

    # chunk start position of each (q, tile)
    tile_chunk_base = np.zeros((4, NT), np.int64)
    for qq in range(4):
        tile_chunk_base[qq] = chunk_base_q[qq] + np.r_[0, np.cumsum(kqt[qq])[:-1]]

    # schedule rows for _build: per chunk -> (q, tile, first, last)
    sched = []
    for qq in range(4):
        for t in range(NT):
            kk = int(kqt[qq, t])
            for j in range(kk):
                sched.append((qq, t, j == 0, j == kk - 1))

    # per-core stream fill
    order = np.lexsort((dloc, tile, q, core))
    src_s = trow[order]
    q_s, t_s, dloc_s = q[order], tile[order], dloc[order]
    core_s = core[order]

    gidx_all, dloc_all = [], []
    for c in range(NC):
        m = core_s == c
        gq, gt, gd, gs = q_s[m], t_s[m], dloc_s[m], src_s[m]
        # rank within (q, tile)
        key = gq * NT + gt
        # edges are sorted by (q, tile, dloc); rank = arange - group start
        starts = np.r_[0, np.flatnonzero(np.diff(key)) + 1]
        grp_start = np.zeros(len(key), np.int64)
        grp_start[starts] = starts
        grp_start = np.maximum.accumulate(grp_start)
        rank = np.arange(len(key)) - grp_start
        pos = tile_chunk_base[gq, gt] * CHK + rank
        gidx = np.zeros(strm, np.int64)
        dlv = np.full(strm, -1.0, np.float32)
        gidx[pos] = gs
        dlv[pos] = gd
        gidx_all.append(_wrap16(gidx))
        dloc_all.append(
            np.ascontiguousarray(
                dlv.reshape(nchunks, CHK).T.astype(BF16)))   # [128, nchunks]

    # weights
    iwcat1 = np.concatenate([iw1[0], iw1[1]], axis=1)        # [512, 32]
    rwcat1 = np.concatenate([rw1[0], rw1[1]], axis=1)        # [512, 32]
    w1 = np.ascontiguousarray(
        np.concatenate([iwcat1, rwcat1], axis=1)).astype(BF16)  # [512, 64]
    b1r = np.tile(np.concatenate([b1[0, 0], b1[1, 0]])[None, :], (128, 1)).astype(np.float32)
    w2 = np.zeros((32, 80), np.float32)
    for k in range(K):
        w2[0:16, 40 * k:40 * k + 40] = iw2[k]
        w2[16:32, 40 * k:40 * k + 40] = rw2[k]
    b2r = np.tile(np.concatenate([b2[0, 0], b2[1, 0]])[None, :], (128, 1)).astype(np.float32)

    iota = np.broadcast_to(
        np.tile(np.arange(128, dtype=np.float32)[None, :], (128, 1)).astype(BF16)[:, None, :],
        (128, SELB, 128)).copy()

    in_maps = []
    for c in range(NC):
        xT = np.zeros((FIN, SP), np.float32)
        xT[:, :S] = x[c * S:(c + 1) * S].T
        dv = np.zeros((128, NT), np.float32)
        dvp = np.zeros(SP, np.float32)
        dvp[:S] = dinv[c * S:(c + 1) * S]
        dv[:, :] = dvp.reshape(NT, 128).T
        in_maps.append({
            "xT": np.ascontiguousarray(xT.astype(BF16)),
            "gidx": gidx_all[c],
            "dloc": dloc_all[c],
            "iota": iota,
            "dinv_t": dv,
            "w1": w1,
            "b1r": b1r,
            "w2": w2,
            "b2r": b2r,
        })
    meta = {"calls": calls, "sched": sched, "nchunks": nchunks, "strm": strm}
    return in_maps, meta


def _build(meta):
    calls = meta["calls"]
    sched = meta["sched"]
    nchunks = meta["nchunks"]
    strm = meta["strm"]

    nc = bacc.Bacc("TRN2", target_bir_lowering=False, num_devices=NC)
    dt = mybir.dt
    f32 = dt.float32
    bf16 = dt.bfloat16

    xT_p = nc.declare_dram_parameter("xT", [FIN, SP], bf16, isOutput=False)
    gidx_p = nc.declare_dram_parameter("gidx", [128, strm // 16], dt.int16, isOutput=False)
    dloc_p = nc.declare_dram_parameter("dloc", [128, nchunks], bf16, isOutput=False)
    iota_p = nc.declare_dram_parameter("iota", [128, SELB, 128], bf16, isOutput=False)
    dinv_p = nc.declare_dram_parameter("dinv_t", [128, NT], f32, isOutput=False)
    w1_p = nc.declare_dram_parameter("w1", [FIN, 64], bf16, isOutput=False)
    b1r_p = nc.declare_dram_parameter("b1r", [128, 32], f32, isOutput=False)
    w2_p = nc.declare_dram_parameter("w2", [32, 80], f32, isOutput=False)
    b2r_p = nc.declare_dram_parameter("b2r", [128, 80], f32, isOutput=False)
    out_p = nc.declare_dram_parameter("out", [S, CLS], f32, isOutput=True)

    ag1_in = nc.dram_tensor("ag1_in", [SP, 128], bf16)
    ag1_out = nc.dram_tensor("ag1_out", [TBL, 128], bf16, addr_space="Shared")
    ag2_in = nc.dram_tensor("ag2_in", [SP, 128], bf16)
    ag2_out = nc.dram_tensor("ag2_out", [TBL, 128], bf16, addr_space="Shared")

    gs = [nc.alloc_semaphore(f"gs{i}") for i in range(NS)]
    cc_sem = nc.alloc_semaphore("cc_sem")
    rg = [list(range(NC))]

    with TileContext(nc) as tc:
        with (
            tc.tile_pool(name="const", bufs=1) as cp,
            tc.tile_pool(name="work", bufs=3) as wp,
            tc.tile_pool(name="edge", bufs=1) as ep,
            tc.tile_pool(name="psum", bufs=2, space="PSUM") as pp,
            tc.tile_pool(name="psum1", bufs=2, space="PSUM") as pp1,
            tc.tile_pool(name="selp", bufs=4) as sp,
        ):
            # ---- resident tiles ----
            gidx_sb = cp.tile([128, strm // 16], dt.int16)
            nc.sync.dma_start(gidx_sb[:], gidx_p[:])
            dloc_sb = cp.tile([128, nchunks], bf16)
            nc.sync.dma_start(dloc_sb[:], dloc_p[:])
            iota_sb = cp.tile([128, SELB, 128], bf16)
            nc.sync.dma_start(iota_sb[:], iota_p[:])
            dinv_sb = cp.tile([128, NT], f32)
            nc.sync.dma_start(dinv_sb[:], dinv_p[:])
            w1_sb = cp.tile([128, 4, 64], bf16)
            nc.sync.dma_start(w1_sb[:], w1_p[:].rearrange("(k p) n -> p k n", p=128))
            b1r_sb = cp.tile([128, 32], f32)
            nc.sync.dma_start(b1r_sb[:], b1r_p[:])
            w2_sb = cp.tile([32, 80], f32)
            nc.sync.dma_start(w2_sb[:], w2_p[:])
            b2r_sb = cp.tile([128, 80], f32)
            nc.sync.dma_start(b2r_sb[:], b2r_p[:])
            ident = cp.tile([128, 128], f32)
            make_identity(nc, ident[:])
            r1_res = cp.tile([128, NT * 32], f32)
            h_res = cp.tile([128, NT * 16], f32)
            agg1 = cp.tile([128, NT, 32], f32)
            agg2 = cp.tile([128, NT, 16], f32)
            scratch = cp.tile([128, 32], bf16)

            msgs = [ep.tile([128, BLK // CHK, 128], bf16, name=f"msg{i}")
                    for i in range(NBUF)]

            # ---- stage 1: projections x @ [iwcat|rwcat] -> L1 table ----
            for t in range(NT):
                xt = wp.tile([128, 4, 128], bf16, tag="xt")
                nc.sync.dma_start(
                    xt[:], xT_p[:, t * 128:(t + 1) * 128].rearrange("(k p) m -> p k m", p=128))
                hps = pp.tile([128, 64], f32, space="PSUM", tag="hps")
                for k in range(4):
                    nc.tensor.matmul(hps[:], lhsT=xt[:, k, :], rhs=w1_sb[:, k, :],
                                     start=(k == 0), stop=(k == 3))
                h1s = wp.tile([128, 32], bf16, tag="h1s")
                nc.vector.tensor_tensor(
                    out=h1s[:], in0=hps[:, 0:32],
                    in1=dinv_sb[:, t:t + 1].to_broadcast([128, 32]),
                    op=mybir.AluOpType.mult)
                nc.sync.dma_start(ag1_in[t * 128:(t + 1) * 128, 0:32], h1s[:])
                nc.scalar.copy(r1_res[:, 32 * t:32 * t + 32], hps[:, 32:64])

            # ---- stage 2: AllGather L1 table ----
            nc.gpsimd.collective_compute(
                "AllGather", mybir.AluOpType.bypass, replica_groups=rg,
                ins=[ag1_in[:].opt()], outs=[ag1_out[:].opt()])

            # ---- edge phase (shared for both layers) ----
            # chunk ci -> call index
            chunk_call = np.zeros(nchunks, np.int64)
            for j, (qq, coff, nch) in enumerate(calls):
                chunk_call[coff:coff + nch] = j

            # counters persist across the two passes
            cnts = {"g": [0] * NS, "pass": 0}

            def edge_pass(table, agg, fw, epi_cb=None):
                """fw: feature width of rhs slice (32 for L1, 16 for L2).
                epi_cb(t) is emitted right after tile t's final drain so its
                compute overlaps the remaining edge-phase tail."""
                cnts["pass"] += 1
                pn = cnts["pass"]
                ncalls = len(calls)
                gthr = {}

                def issue_call(j):
                    """Issue one gather call (desc-gen on gpsimd)."""
                    with tc.tile_critical():
                        if j == 0:
                            # bind collective completion (custom DMA can't
                            # carry walrus waits): probe-read the AG output.
                            nc.gpsimd.memset(scratch[:], 0.0)
                            nc.gpsimd.dma_start(
                                scratch[0:1, 0:32],
                                table[0:1, 0:32]).then_inc(cc_sem, 16)
                            nc.gpsimd.wait_ge(cc_sem, 16 * pn)
                            nc.gpsimd.load_library(mlp_lib)
                        qq, coff, nch = calls[j]
                        eoff = coff * CHK
                        nidx = nch * CHK
                        cnts["g"][j % NS] += 1
                        gthr[j] = cnts["g"][j % NS]
                        nc.gpsimd.dma_gather(
                            out_ap=msgs[j % NBUF][:, 0:nch, :],
                            in_ap=table[QR * qq:QR * (qq + 1), :],
                            idxs_ap=gidx_sb[:, eoff // 16:(eoff + nidx) // 16],
                            num_idxs=nidx, num_idxs_reg=nidx, elem_size=128,
                        ).then_inc(gs[j % NS], 16)

                def finish_call(j):
                    """Wait (on the idle Scalar engine) for call j's gather
                    DMA to land, then re-mark its buffer as written (tiny
                    write into the unread junk column zone) so Tile orders
                    consumers after the data actually landed. Must sit in a
                    tile_critical: outside one, Tile's scheduling sim does
                    not credit the DMA-completion sem and reports deadlock."""
                    with tc.tile_critical():
                        nc.scalar.wait_ge(gs[j % NS], 16 * gthr[j])
                        nc.scalar.memzero(msgs[j % NBUF][0:1, 0:1, 64:66])

                # Tile-managed consumption; desc-gen runs ahead on gpsimd,
                # bounded by msg-buffer WAR deps; completion waits live on
                # the Scalar queue so gpsimd never stalls on DMA tails.
                issued = 0
                finished = 0
                pst = None
                for ci, (qq, t, first, last) in enumerate(sched):
                    j = int(chunk_call[ci])
                    # issue gather calls ahead of consumption; the bound
                    # guarantees no buffer can overwrite data whose reader
                    # instructions aren't emitted yet.
                    while issued < ncalls and issued <= j + NBUF - 1:
                        issue_call(issued)
                        issued += 1
                    while finished <= j:
                        finish_call(finished)
                        finished += 1
                    if ci % SELB == 0:
                        nsel = min(SELB, nchunks - ci)
                        sel8 = sp.tile([128, SELB, 128], bf16, tag="sel")
                        nc.vector.tensor_tensor(
                            out=sel8[:, 0:nsel, :],
                            in0=iota_sb[:, 0:nsel, :],
                            in1=dloc_sb[:, ci:ci + nsel].to_broadcast(
                                [128, nsel, 128]),
                            op=mybir.AluOpType.is_equal)
                    if first:
                        pst = pp1.tile([128, fw], f32, space="PSUM", tag="eps")
                    cin = ci - calls[j][1]
                    nc.tensor.matmul(
                        pst[:], lhsT=sel8[:, ci % SELB, :],
                        rhs=msgs[j % NBUF][:, cin, 0:fw],
                        start=first, stop=last)
                    if last:
                        sl = agg[:, t, :]
                        if qq == 0:
                            nc.vector.tensor_scalar(
                                out=sl, in0=pst[:], scalar1=1.0,
                                scalar2=None, op0=mybir.AluOpType.mult)
                        else:
                            nc.vector.tensor_tensor(
                                out=sl, in0=pst[:], in1=sl,
                                op=mybir.AluOpType.add)
                        if qq == 3 and epi_cb is not None:
                            epi_cb(t)

            def epi1(t):
                dvb = dinv_sb[:, t:t + 1].to_broadcast([128, 32])
                v = wp.tile([128, 32], f32, tag="v")
                nc.vector.tensor_tensor(out=v[:], in0=agg1[:, t, :], in1=dvb,
                                        op=mybir.AluOpType.mult)
                nc.vector.tensor_tensor(out=v[:], in0=v[:], in1=r1_res[:, 32 * t:32 * t + 32],
                                        op=mybir.AluOpType.add)
                nc.vector.tensor_tensor(out=v[:], in0=v[:], in1=b1r_sb[:],
                                        op=mybir.AluOpType.add)
                nc.vector.tensor_scalar(out=v[:], in0=v[:], scalar1=0.0, scalar2=None,
                                        op0=mybir.AluOpType.max)
                h = h_res[:, 16 * t:16 * t + 16]
                nc.vector.tensor_tensor(out=h, in0=v[:, 0:16], in1=v[:, 16:32],
                                        op=mybir.AluOpType.add)
                nc.vector.tensor_scalar(out=h, in0=h, scalar1=0.5, scalar2=None,
                                        op0=mybir.AluOpType.mult)
                h2s = wp.tile([128, 16], bf16, tag="h2s")
                nc.vector.tensor_tensor(
                    out=h2s[:], in0=h,
                    in1=dinv_sb[:, t:t + 1].to_broadcast([128, 16]),
                    op=mybir.AluOpType.mult)
                nc.sync.dma_start(ag2_in[t * 128:(t + 1) * 128, 0:16], h2s[:])

            edge_pass(ag1_out, agg1, 32, epi_cb=epi1)

            # ---- AllGather L2 table ----
            nc.gpsimd.collective_compute(
                "AllGather", mybir.AluOpType.bypass, replica_groups=rg,
                ins=[ag2_in[:].opt()], outs=[ag2_out[:].opt()])

            # ---- L2 epilogue (per tile, overlapped via epi_cb) ----
            def epi2(t):
                cc = wp.tile([128, 32], f32, tag="cc")
                nc.vector.tensor_tensor(
                    out=cc[:, 0:16], in0=agg2[:, t, :],
                    in1=dinv_sb[:, t:t + 1].to_broadcast([128, 16]),
                    op=mybir.AluOpType.mult)
                nc.scalar.copy(cc[:, 16:32], h_res[:, 16 * t:16 * t + 16])
                ccT_ps = pp.tile([32, 128], f32, space="PSUM", tag="ccT")
                nc.tensor.transpose(out=ccT_ps[:], in_=cc[:], identity=ident[:])
                ccT = wp.tile([32, 128], f32, tag="ccTs")
                nc.scalar.copy(ccT[:], ccT_ps[:])
                ops = pp.tile([128, 80], f32, space="PSUM", tag="ops")
                nc.tensor.matmul(ops[:], lhsT=ccT[:], rhs=w2_sb[:], start=True, stop=True)
                o = wp.tile([128, 80], f32, tag="o")
                nc.vector.tensor_tensor(out=o[:], in0=ops[:], in1=b2r_sb[:],
                                        op=mybir.AluOpType.add)
                nc.vector.tensor_scalar(out=o[:], in0=o[:], scalar1=0.0, scalar2=None,
                                        op0=mybir.AluOpType.max)
                fin = wp.tile([128, CLS], f32, tag="fin")
                nc.vector.tensor_tensor(out=fin[:], in0=o[:, 0:40], in1=o[:, 40:80],
                                        op=mybir.AluOpType.add)
                nc.vector.tensor_scalar(out=fin[:], in0=fin[:], scalar1=0.5, scalar2=None,
                                        op0=mybir.AluOpType.mult)
                lo = t * 128
                hi = min(lo + 128, S)
                if hi > lo:
                    nc.sync.dma_start(out_p[lo:hi, :], fin[0:hi - lo, :])

            edge_pass(ag2_out, agg2, 16, epi_cb=epi2)

    nc.compile()
    return nc


def _make_runner(nc, n_cores=NC):
    import jax
    from jax.sharding import Mesh, PartitionSpec, NamedSharding
    from jax.experimental.shard_map import shard_map
    from concourse.bass2jax import (
        _bass_exec_p, install_neuronx_cc_hook, partition_id_tensor)

    install_neuronx_cc_hook()
    partition_name = nc.partition_id_tensor.name if nc.partition_id_tensor else None
    in_names, out_names, out_avals, zero_outs = [], [], [], []
    for alloc in nc.m.functions[0].allocations:
        if not isinstance(alloc, mybir.MemoryLocationSet):
            continue
        name = alloc.memorylocations[0].name
        if alloc.kind == "ExternalInput":
            if name != partition_name:
                in_names.append(name)
        elif alloc.kind == "ExternalOutput":
            out_names.append(name)
            shape = tuple(alloc.tensor_shape)
            dtype = mybir.dt.np(alloc.dtype)
            out_avals.append(jax.core.ShapedArray(shape, dtype))
            zero_outs.append(np.zeros(shape, dtype))
    n_params = len(in_names)
    in_names_full = list(in_names) + out_names
    if partition_name is not None:
        in_names_full.append(partition_name)

    def _body(*args):
        operands = list(args)
        if partition_name is not None:
            operands.append(partition_id_tensor())
        outs = _bass_exec_p.bind(
            *operands,
            out_avals=tuple(out_avals),
            in_names=tuple(in_names_full),
            out_names=tuple(out_names),
            lowering_input_output_aliases=(),
            sim_require_finite=True,
            sim_require_nnan=True,
            nc=nc,
        )
        return tuple(outs)

    devices = jax.devices()[:n_cores]
    mesh = Mesh(np.asarray(devices), ("core",))
    in_specs = (PartitionSpec("core"),) * (n_params + len(out_names))
    out_specs = (PartitionSpec("core"),) * len(out_names)
    sharded = jax.jit(
        shard_map(_body, mesh=mesh, in_specs=in_specs, out_specs=out_specs,
                  check_rep=False),
        keep_unused=True)

    def run(in_maps, repeats=1):
        sh = NamedSharding(mesh, PartitionSpec("core"))
        per_core = [[np.asarray(m[k]) for k in in_names] for m in in_maps]
        concat_in = [
            jax.device_put(
                np.concatenate([per_core[c][i] for c in range(n_cores)], axis=0), sh)
            for i in range(n_params)
        ]
        concat_zeros = [
            jax.device_put(
                np.zeros((n_cores * z.shape[0], *z.shape[1:]), z.dtype), sh)
            for z in zero_outs
        ]
        import jax as _j
        _j.block_until_ready(concat_in)
        _j.block_until_ready(concat_zeros)
        times = []
        out_arrs = None
        for _ in range(repeats):
            t0 = time.perf_counter()
            out_arrs = sharded(*concat_in, *concat_zeros)
            _j.block_until_ready(out_arrs)
            times.append(time.perf_counter() - t0)
        results = [
            {name: np.asarray(out_arrs[i]).reshape(n_cores, *out_avals[i].shape)[c]
             for i, name in enumerate(out_names)}
            for c in range(n_cores)
        ]
        return results, times

    return run



def kernel(x, edge_index, iw1, rw1, b1, iw2, rw2, b2, _timing=None, _expose=None):
    x = np.asarray(x, dtype=np.float32)
    edge_index = np.asarray(edge_index)
    in_maps, meta = _preprocess(
        x, edge_index, np.asarray(iw1), np.asarray(rw1), np.asarray(b1),
        np.asarray(iw2), np.asarray(rw2), np.asarray(b2))
    key = ("v3", meta["nchunks"], meta["strm"])
    if key not in _cache:
        nc = _build(meta)
        _cache[key] = (_make_runner(nc), nc)
    run, nc = _cache[key]
    repeats = 30 if _timing is not None else 1
    results, times = run(in_maps, repeats=repeats)
    if _timing is not None:
        _timing.extend(times)
    if _expose is not None:
        _expose.update({"run": run, "in_maps": in_maps, "nc": nc})
    out = np.concatenate([results[c]["out"] for c in range(NC)], axis=0)
    return out



# revision 14
# speedup vs baseline: 1.5868x; 1.5868x over previous
"""ARMA GNN (2-layer, K=2 stacks) distributed Bass kernel for 8 TRN2 NeuronCores.

v2: dst-sorted edge streams + PE selector-matmul segment reduction.
 - Nodes sharded 12500/core; edges partitioned by destination core.
 - Layer math refactored so message passing happens at small feature dims
   (32 cols L1, 16 cols L2), with the stack projections applied before (L1)
   or after (L2) aggregation.
 - The all-gathered per-node table is stored bf16, one node per 256B row.
 - Edge phase per layer: SWDGE dma_gather fetches src rows in dst-sorted
   order (8192 edges/call); for every 128-edge chunk the vector engine
   builds a one-hot selector S^T[j, i] = (dloc[j] == i) and the tensor
   engine accumulates S^T^T @ msg into the dst tile's PSUM bank. No
   scatter descriptors, no conflict packing; f32 PSUM accumulation.
 - agg lives in SBUF; epilogues identical in spirit to v1.
"""
import sys
import time

sys.path.insert(0, "/opt/trn_rl_repo")

import numpy as np
import ml_dtypes

import concourse.bass as bass
import concourse.bacc as bacc
import concourse.mybir as mybir
from concourse.tile import TileContext
from concourse.masks import make_identity
from concourse.library_config import mlp as mlp_lib

BF16 = ml_dtypes.bfloat16

N = 100000
E = 3200000
NC = 8
S = 12500            # nodes per core
NT = 98              # node tiles per core
SP = NT * 128        # 12544 padded nodes per core
QR = 2 * SP          # table rows per quarter (2 core shards)
TBL = NC * SP        # all-gathered table rows
BLK = 1024           # edges per gather call (SWDGE ucode limit)
CHK = 128            # edges per selector chunk
CPC = 16             # gather calls per critical section
NBUF = 32            # rotating msg buffers (>= 2*CPC)
NS = 8               # rotating gather-dma semaphores
FIN, HID, CLS, K = 512, 16, 40, 2

_cache = {}


def _wrap16(idx):
    """[n] int -> [128, n//16] int16: pos i at [i%16, i//16], replicated x8."""
    n = idx.shape[0]
    w = idx.astype(np.int16).reshape(n // 16, 16).T
    return np.ascontiguousarray(np.tile(w, (8, 1)))


def _preprocess(x, edge_index, iw1, rw1, b1, iw2, rw2, b2):
    src = edge_index[0].astype(np.int64)
    dst = edge_index[1].astype(np.int64)
    deg = np.bincount(dst, minlength=N).astype(np.float32)
    dinv = np.where(deg > 0, 1.0 / np.sqrt(deg), 0.0).astype(np.float32)

    core = dst // S
    q = src // (2 * S)                      # source quarter
    trow = (src // S) * SP + (src % S) - QR * q   # quarter-relative table row
    dl = dst - core * S                     # local dst row [0,12500)
    tile = dl // 128
    dloc = dl % 128

    # global (core-uniform) chunk schedule: chunks per (q, tile)
    cnt = np.zeros((NC, 4, NT), np.int64)
    np.add.at(cnt, (core, q, tile), 1)
    kqt = np.ceil(cnt.max(axis=0) / CHK).astype(np.int64)      # [4, NT]
    kqt = np.maximum(kqt, 3)   # >=3 also guards the ds-wait/ceil4 deadlock
    nchunks_q = kqt.sum(axis=1)                                # per quarter
    # call layout per quarter: calls of <=64 chunks
    calls = []                       # list of (q, chunk_off_in_stream, nchunks)
    chunk_base_q = np.zeros(4, np.int64)
    off = 0
    for qq in range(4):
        chunk_base_q[qq] = off
        rem = int(nchunks_q[qq])
        pos = off
        while rem > 0:
            take = min(BLK // CHK, rem)
            calls.append((qq, pos, take))
            pos += take
            rem -= take
        off += int(nchunks_q[qq])
    nchunks = int(off)
    strm = nchunks * CHK

    # chunk start position of each (q, tile)
    tile_chunk_base = np.zeros((4, NT), np.int64)
    for qq in range(4):
        tile_chunk_base[qq] = chunk_base_q[qq] + np.r_[0, np.cumsum(kqt[qq])[:-1]]

    # schedule rows for _build: per chunk -> (q, tile, first, last)
    sched = []
    for qq in range(4):
        for t in range(NT):
            kk = int(kqt[qq, t])
            for j in range(kk):
                sched.append((qq, t, j == 0, j == kk - 1))

    # per-core stream fill
    order = np.lexsort((dloc, tile, q, core))
    src_s = trow[order]
    q_s, t_s, dloc_s = q[order], tile[order], dloc[order]
    core_s = core[order]

    gidx_all, dloc_all = [], []
    for c in range(NC):
        m = core_s == c
        gq, gt, gd, gs = q_s[m], t_s[m], dloc_s[m], src_s[m]
        # rank within (q, tile)
        key = gq * NT + gt
        # edges are sorted by (q, tile, dloc); rank = arange - group start
        starts = np.r_[0, np.flatnonzero(np.diff(key)) + 1]
        grp_start = np.zeros(len(key), np.int64)
        grp_start[starts] = starts
        grp_start = np.maximum.accumulate(grp_start)
        rank = np.arange(len(key)) - grp_start
        pos = tile_chunk_base[gq, gt] * CHK + rank
        gidx = np.zeros(strm, np.int64)
        dlv = np.full(strm, -1.0, np.float32)
        gidx[pos] = gs
        dlv[pos] = gd
        gidx_all.append(_wrap16(gidx))
        dloc_all.append(
            np.ascontiguousarray(
                dlv.reshape(nchunks, CHK).T.astype(BF16)))   # [128, nchunks]

    # weights
    iwcat1 = np.concatenate([iw1[0], iw1[1]], axis=1)        # [512, 32]
    rwcat1 = np.concatenate([rw1[0], rw1[1]], axis=1)        # [512, 32]
    w1 = np.ascontiguousarray(
        np.concatenate([iwcat1, rwcat1], axis=1)).astype(BF16)  # [512, 64]
    b1r = np.tile(np.concatenate([b1[0, 0], b1[1, 0]])[None, :], (128, 1)).astype(np.float32)
    w2 = np.zeros((32, 80), np.float32)
    for k in range(K):
        w2[0:16, 40 * k:40 * k + 40] = iw2[k]
        w2[16:32, 40 * k:40 * k + 40] = rw2[k]
    b2r = np.tile(np.concatenate([b2[0, 0], b2[1, 0]])[None, :], (128, 1)).astype(np.float32)

    iota = np.tile(np.arange(128, dtype=np.float32)[None, :], (128, 1)).astype(BF16)

    in_maps = []
    for c in range(NC):
        xT = np.zeros((FIN, SP), np.float32)
        xT[:, :S] = x[c * S:(c + 1) * S].T
        dv = np.zeros((128, NT), np.float32)
        dvp = np.zeros(SP, np.float32)
        dvp[:S] = dinv[c * S:(c + 1) * S]
        dv[:, :] = dvp.reshape(NT, 128).T
        in_maps.append({
            "xT": np.ascontiguousarray(xT.astype(BF16)),
            "gidx": gidx_all[c],
            "dloc": dloc_all[c],
            "iota": iota,
            "dinv_t": dv,
            "w1": w1,
            "b1r": b1r,
            "w2": w2,
            "b2r": b2r,
        })
    meta = {"calls": calls, "sched": sched, "nchunks": nchunks, "strm": strm}
    return in_maps, meta


def _build(meta):
    calls = meta["calls"]
    sched = meta["sched"]
    nchunks = meta["nchunks"]
    strm = meta["strm"]

    nc = bacc.Bacc("TRN2", target_bir_lowering=False, num_devices=NC)
    dt = mybir.dt
    f32 = dt.float32
    bf16 = dt.bfloat16

    xT_p = nc.declare_dram_parameter("xT", [FIN, SP], bf16, isOutput=False)
    gidx_p = nc.declare_dram_parameter("gidx", [128, strm // 16], dt.int16, isOutput=False)
    dloc_p = nc.declare_dram_parameter("dloc", [128, nchunks], bf16, isOutput=False)
    iota_p = nc.declare_dram_parameter("iota", [128, 128], bf16, isOutput=False)
    dinv_p = nc.declare_dram_parameter("dinv_t", [128, NT], f32, isOutput=False)
    w1_p = nc.declare_dram_parameter("w1", [FIN, 64], bf16, isOutput=False)
    b1r_p = nc.declare_dram_parameter("b1r", [128, 32], f32, isOutput=False)
    w2_p = nc.declare_dram_parameter("w2", [32, 80], f32, isOutput=False)
    b2r_p = nc.declare_dram_parameter("b2r", [128, 80], f32, isOutput=False)
    out_p = nc.declare_dram_parameter("out", [S, CLS], f32, isOutput=True)

    ag1_in = nc.dram_tensor("ag1_in", [SP, 128], bf16)
    ag1_out = nc.dram_tensor("ag1_out", [TBL, 128], bf16, addr_space="Shared")
    ag2_in = nc.dram_tensor("ag2_in", [SP, 128], bf16)
    ag2_out = nc.dram_tensor("ag2_out", [TBL, 128], bf16, addr_space="Shared")

    gs = [nc.alloc_semaphore(f"gs{i}") for i in range(NS)]
    cc_sem = nc.alloc_semaphore("cc_sem")
    rg = [list(range(NC))]

    with TileContext(nc) as tc:
        with (
            tc.tile_pool(name="const", bufs=1) as cp,
            tc.tile_pool(name="work", bufs=3) as wp,
            tc.tile_pool(name="edge", bufs=1) as ep,
            tc.tile_pool(name="psum", bufs=2, space="PSUM") as pp,
            tc.tile_pool(name="psum1", bufs=2, space="PSUM") as pp1,
            tc.tile_pool(name="selp", bufs=8) as sp,
        ):
            # ---- resident tiles ----
            gidx_sb = cp.tile([128, strm // 16], dt.int16)
            nc.sync.dma_start(gidx_sb[:], gidx_p[:])
            dloc_sb = cp.tile([128, nchunks], bf16)
            nc.sync.dma_start(dloc_sb[:], dloc_p[:])
            iota_sb = cp.tile([128, 128], bf16)
            nc.sync.dma_start(iota_sb[:], iota_p[:])
            dinv_sb = cp.tile([128, NT], f32)
            nc.sync.dma_start(dinv_sb[:], dinv_p[:])
            w1_sb = cp.tile([128, 4, 64], bf16)
            nc.sync.dma_start(w1_sb[:], w1_p[:].rearrange("(k p) n -> p k n", p=128))
            b1r_sb = cp.tile([128, 32], f32)
            nc.sync.dma_start(b1r_sb[:], b1r_p[:])
            w2_sb = cp.tile([32, 80], f32)
            nc.sync.dma_start(w2_sb[:], w2_p[:])
            b2r_sb = cp.tile([128, 80], f32)
            nc.sync.dma_start(b2r_sb[:], b2r_p[:])
            ident = cp.tile([128, 128], f32)
            make_identity(nc, ident[:])
            r1_res = cp.tile([128, NT * 32], f32)
            h_res = cp.tile([128, NT * 16], f32)
            agg1 = cp.tile([128, NT, 32], f32)
            agg2 = cp.tile([128, NT, 16], f32)
            scratch = cp.tile([128, 32], bf16)

            msgs = [ep.tile([128, BLK // CHK, 128], bf16, name=f"msg{i}")
                    for i in range(NBUF)]

            # ---- stage 1: projections x @ [iwcat|rwcat] -> L1 table ----
            for t in range(NT):
                xt = wp.tile([128, 4, 128], bf16, tag="xt")
                nc.sync.dma_start(
                    xt[:], xT_p[:, t * 128:(t + 1) * 128].rearrange("(k p) m -> p k m", p=128))
                hps = pp.tile([128, 64], f32, space="PSUM", tag="hps")
                for k in range(4):
                    nc.tensor.matmul(hps[:], lhsT=xt[:, k, :], rhs=w1_sb[:, k, :],
                                     start=(k == 0), stop=(k == 3))
                h1s = wp.tile([128, 32], bf16, tag="h1s")
                nc.vector.tensor_tensor(
                    out=h1s[:], in0=hps[:, 0:32],
                    in1=dinv_sb[:, t:t + 1].to_broadcast([128, 32]),
                    op=mybir.AluOpType.mult)
                nc.sync.dma_start(ag1_in[t * 128:(t + 1) * 128, 0:32], h1s[:])
                nc.scalar.copy(r1_res[:, 32 * t:32 * t + 32], hps[:, 32:64])

            # ---- stage 2: AllGather L1 table ----
            nc.gpsimd.collective_compute(
                "AllGather", mybir.AluOpType.bypass, replica_groups=rg,
                ins=[ag1_in[:].opt()], outs=[ag1_out[:].opt()])

            # ---- edge phase (shared for both layers) ----
            # chunk ci -> call index
            chunk_call = np.zeros(nchunks, np.int64)
            for j, (qq, coff, nch) in enumerate(calls):
                chunk_call[coff:coff + nch] = j

            # counters persist across the two passes
            cnts = {"g": [0] * NS, "pass": 0}

            def edge_pass(table, agg, fw, epi_cb=None):
                """fw: feature width of rhs slice (32 for L1, 16 for L2).
                epi_cb(t) is emitted right after tile t's final drain so its
                compute overlaps the remaining edge-phase tail."""
                cnts["pass"] += 1
                pn = cnts["pass"]
                ncalls = len(calls)
                gthr = {}

                def finish_group(lo, hi):
                    """Wait for group [lo,hi)'s gather DMAs and re-mark their
                    buffers as written (a 1-elem write into the unread junk
                    column zone) so Tile orders consumers after THIS critical,
                    i.e. after the data actually landed."""
                    for j in range(lo, hi):
                        nc.gpsimd.wait_ge(gs[j % NS], 16 * gthr[j])
                    for j in range(lo, hi):
                        nc.gpsimd.memset(msgs[j % NBUF][0:1, 0:1, 64:65], 0.0)

                def gather_group(cg):
                    """One critical: issue gathers for calls cg..cg+CPC-1;
                    completion of the PREVIOUS group is waited here, so its
                    DMA tail hides behind this group's descriptor generation."""
                    lo, hi = cg, min(cg + CPC, ncalls)
                    with tc.tile_critical():
                        if cg == 0:
                            # bind collective completion (custom DMA can't
                            # carry walrus waits): probe-read the AG output.
                            nc.gpsimd.memset(scratch[:], 0.0)
                            nc.gpsimd.dma_start(
                                scratch[0:1, 0:32],
                                table[0:1, 0:32]).then_inc(cc_sem, 16)
                            nc.gpsimd.wait_ge(cc_sem, 16 * pn)
                            nc.gpsimd.load_library(mlp_lib)
                        for j in range(lo, hi):
                            qq, coff, nch = calls[j]
                            eoff = coff * CHK
                            nidx = nch * CHK
                            cnts["g"][j % NS] += 1
                            gthr[j] = cnts["g"][j % NS]
                            nc.gpsimd.dma_gather(
                                out_ap=msgs[j % NBUF][:, 0:nch, :],
                                in_ap=table[QR * qq:QR * (qq + 1), :],
                                idxs_ap=gidx_sb[:, eoff // 16:(eoff + nidx) // 16],
                                num_idxs=nidx, num_idxs_reg=nidx, elem_size=128,
                            ).then_inc(gs[j % NS], 16)
                        if cg > 0:
                            finish_group(cg - CPC, cg)

                def tail_group(cg):
                    with tc.tile_critical():
                        finish_group(cg, ncalls)

                # Tile-managed consumption; gather criticals interleave so
                # the scheduler can overlap desc-gen with PE/DVE consumption.
                issued = 0
                pst = None
                last_lo = ((ncalls - 1) // CPC) * CPC
                tail_done = False
                for ci, (qq, t, first, last) in enumerate(sched):
                    j = int(chunk_call[ci])
                    # issue gather groups ahead of consumption; the bound
                    # guarantees no buffer in the group can overwrite data
                    # whose reader instructions aren't emitted yet.
                    while issued < ncalls and issued <= j + NBUF - CPC:
                        gather_group(issued)
                        issued += CPC
                    if j >= last_lo and not tail_done:
                        tail_done = True
                        tail_group(last_lo)
                    sel = sp.tile([128, 128], bf16, tag="sel")
                    nc.vector.tensor_tensor(
                        out=sel[:], in0=iota_sb[:],
                        in1=dloc_sb[:, ci:ci + 1].to_broadcast([128, 128]),
                        op=mybir.AluOpType.is_equal)
                    if first:
                        pst = pp1.tile([128, fw], f32, space="PSUM", tag="eps")
                    cin = ci - calls[j][1]
                    nc.tensor.matmul(
                        pst[:], lhsT=sel[:],
                        rhs=msgs[j % NBUF][:, cin, 0:fw],
                        start=first, stop=last)
                    if last:
                        sl = agg[:, t, :]
                        if qq == 0:
                            nc.vector.tensor_scalar(
                                out=sl, in0=pst[:], scalar1=1.0,
                                scalar2=None, op0=mybir.AluOpType.mult)
                        else:
                            nc.vector.tensor_tensor(
                                out=sl, in0=pst[:], in1=sl,
                                op=mybir.AluOpType.add)
                        if qq == 3 and epi_cb is not None:
                            epi_cb(t)

            def epi1(t):
                dvb = dinv_sb[:, t:t + 1].to_broadcast([128, 32])
                v = wp.tile([128, 32], f32, tag="v")
                nc.vector.tensor_tensor(out=v[:], in0=agg1[:, t, :], in1=dvb,
                                        op=mybir.AluOpType.mult)
                nc.vector.tensor_tensor(out=v[:], in0=v[:], in1=r1_res[:, 32 * t:32 * t + 32],
                                        op=mybir.AluOpType.add)
                nc.vector.tensor_tensor(out=v[:], in0=v[:], in1=b1r_sb[:],
                                        op=mybir.AluOpType.add)
                nc.vector.tensor_scalar(out=v[:], in0=v[:], scalar1=0.0, scalar2=None,
                                        op0=mybir.AluOpType.max)
                h = h_res[:, 16 * t:16 * t + 16]
                nc.vector.tensor_tensor(out=h, in0=v[:, 0:16], in1=v[:, 16:32],
                                        op=mybir.AluOpType.add)
                nc.vector.tensor_scalar(out=h, in0=h, scalar1=0.5, scalar2=None,
                                        op0=mybir.AluOpType.mult)
                h2s = wp.tile([128, 16], bf16, tag="h2s")
                nc.vector.tensor_tensor(
                    out=h2s[:], in0=h,
                    in1=dinv_sb[:, t:t + 1].to_broadcast([128, 16]),
                    op=mybir.AluOpType.mult)
                nc.sync.dma_start(ag2_in[t * 128:(t + 1) * 128, 0:16], h2s[:])

            edge_pass(ag1_out, agg1, 32, epi_cb=epi1)

            # ---- AllGather L2 table ----
            nc.gpsimd.collective_compute(
                "AllGather", mybir.AluOpType.bypass, replica_groups=rg,
                ins=[ag2_in[:].opt()], outs=[ag2_out[:].opt()])

            # ---- L2 epilogue (per tile, overlapped via epi_cb) ----
            def epi2(t):
                cc = wp.tile([128, 32], f32, tag="cc")
                nc.vector.tensor_tensor(
                    out=cc[:, 0:16], in0=agg2[:, t, :],
                    in1=dinv_sb[:, t:t + 1].to_broadcast([128, 16]),
                    op=mybir.AluOpType.mult)
                nc.scalar.copy(cc[:, 16:32], h_res[:, 16 * t:16 * t + 16])
                ccT_ps = pp.tile([32, 128], f32, space="PSUM", tag="ccT")
                nc.tensor.transpose(out=ccT_ps[:], in_=cc[:], identity=ident[:])
                ccT = wp.tile([32, 128], f32, tag="ccTs")
                nc.scalar.copy(ccT[:], ccT_ps[:])
                ops = pp.tile([128, 80], f32, space="PSUM", tag="ops")
                nc.tensor.matmul(ops[:], lhsT=ccT[:], rhs=w2_sb[:], start=True, stop=True)
                o = wp.tile([128, 80], f32, tag="o")
                nc.vector.tensor_tensor(out=o[:], in0=ops[:], in1=b2r_sb[:],
                                        op=mybir.AluOpType.add)
                nc.vector.tensor_scalar(out=o[:], in0=o[:], scalar1=0.0, scalar2=None,
                                        op0=mybir.AluOpType.max)
                fin = wp.tile([128, CLS], f32, tag="fin")
                nc.vector.tensor_tensor(out=fin[:], in0=o[:, 0:40], in1=o[:, 40:80],
                                        op=mybir.AluOpType.add)
                nc.vector.tensor_scalar(out=fin[:], in0=fin[:], scalar1=0.5, scalar2=None,
                                        op0=mybir.AluOpType.mult)
                lo = t * 128
                hi = min(lo + 128, S)
                if hi > lo:
                    nc.sync.dma_start(out_p[lo:hi, :], fin[0:hi - lo, :])

            edge_pass(ag2_out, agg2, 16, epi_cb=epi2)

    nc.compile()
    return nc


def _make_runner(nc, n_cores=NC):
    import jax
    from jax.sharding import Mesh, PartitionSpec, NamedSharding
    from jax.experimental.shard_map import shard_map
    from concourse.bass2jax import (
        _bass_exec_p, install_neuronx_cc_hook, partition_id_tensor)

    install_neuronx_cc_hook()
    partition_name = nc.partition_id_tensor.name if nc.partition_id_tensor else None
    in_names, out_names, out_avals, zero_outs = [], [], [], []
    for alloc in nc.m.functions[0].allocations:
        if not isinstance(alloc, mybir.MemoryLocationSet):
            continue
        name = alloc.memorylocations[0].name
        if alloc.kind == "ExternalInput":
            if name != partition_name:
                in_names.append(name)
        elif alloc.kind == "ExternalOutput":
            out_names.append(name)
            shape = tuple(alloc.tensor_shape)
            dtype = mybir.dt.np(alloc.dtype)
            out_avals.append(jax.core.ShapedArray(shape, dtype))
            zero_outs.append(np.zeros(shape, dtype))
    n_params = len(in_names)
    in_names_full = list(in_names) + out_names
    if partition_name is not None:
        in_names_full.append(partition_name)

    def _body(*args):
        operands = list(args)
        if partition_name is not None:
            operands.append(partition_id_tensor())
        outs = _bass_exec_p.bind(
            *operands,
            out_avals=tuple(out_avals),
            in_names=tuple(in_names_full),
            out_names=tuple(out_names),
            lowering_input_output_aliases=(),
            sim_require_finite=True,
            sim_require_nnan=True,
            nc=nc,
        )
        return tuple(outs)

    devices = jax.devices()[:n_cores]
    mesh = Mesh(np.asarray(devices), ("core",))
    in_specs = (PartitionSpec("core"),) * (n_params + len(out_names))
    out_specs = (PartitionSpec("core"),) * len(out_names)
    sharded = jax.jit(
        shard_map(_body, mesh=mesh, in_specs=in_specs, out_specs=out_specs,
                  check_rep=False),
        keep_unused=True)

    def run(in_maps, repeats=1):
        sh = NamedSharding(mesh, PartitionSpec("core"))
        per_core = [[np.asarray(m[k]) for k in in_names] for m in in_maps]
        concat_in = [
            jax.device_put(
                np.concatenate([per_core[c][i] for c in range(n_cores)], axis=0), sh)
            for i in range(n_params)
        ]
        concat_zeros = [
            jax.device_put(
                np.zeros((n_cores * z.shape[0], *z.shape[1:]), z.dtype), sh)
            for z in zero_outs
        ]
        import jax as _j
        _j.block_until_ready(concat_in)
        _j.block_until_ready(concat_zeros)
        times = []
        out_arrs = None
        for _ in range(repeats):
            t0 = time.perf_counter()
            out_arrs = sharded(*concat_in, *concat_zeros)
            _j.block_until_ready(out_arrs)
            times.append(time.perf_counter() - t0)
        results = [
            {name: np.asarray(out_arrs[i]).reshape(n_cores, *out_avals[i].shape)[c]
             for i, name in enumerate(out_names)}
            for c in range(n_cores)
        ]
        return results, times

    return run



def kernel(x, edge_index, iw1, rw1, b1, iw2, rw2, b2, _timing=None, _expose=None):
    x = np.asarray(x, dtype=np.float32)
    edge_index = np.asarray(edge_index)
    in_maps, meta = _preprocess(
        x, edge_index, np.asarray(iw1), np.asarray(rw1), np.asarray(b1),
        np.asarray(iw2), np.asarray(rw2), np.asarray(b2))
    key = ("v2", meta["nchunks"], meta["strm"])
    if key not in _cache:
        nc = _build(meta)
        _cache[key] = (_make_runner(nc), nc)
    run, nc = _cache[key]
    repeats = 30 if _timing is not None else 1
    results, times = run(in_maps, repeats=repeats)
    if _timing is not None:
        _timing.extend(times)
    if _expose is not None:
        _expose.update({"run": run, "in_maps": in_maps, "nc": nc})
    out = np.concatenate([results[c]["out"] for c in range(NC)], axis=0)
    return out



# revision 15
# speedup vs baseline: 2.3320x; 1.4696x over previous
"""ARMA GNN (2-layer, K=2 stacks) distributed Bass kernel for 8 TRN2 NeuronCores.

v2: dst-sorted edge streams + PE selector-matmul segment reduction.
 - Nodes sharded 12500/core; edges partitioned by destination core.
 - Layer math refactored so message passing happens at small feature dims
   (32 cols L1, 16 cols L2), with the stack projections applied before (L1)
   or after (L2) aggregation.
 - The all-gathered per-node table is stored bf16, one node per 256B row.
 - Edge phase per layer: SWDGE dma_gather fetches src rows in dst-sorted
   order (8192 edges/call); for every 128-edge chunk the vector engine
   builds a one-hot selector S^T[j, i] = (dloc[j] == i) and the tensor
   engine accumulates S^T^T @ msg into the dst tile's PSUM bank. No
   scatter descriptors, no conflict packing; f32 PSUM accumulation.
 - agg lives in SBUF; epilogues identical in spirit to v1.
"""
import sys
import time

sys.path.insert(0, "/opt/trn_rl_repo")

import numpy as np
import ml_dtypes

import concourse.bass as bass
import concourse.bacc as bacc
import concourse.mybir as mybir
from concourse.tile import TileContext
from concourse.masks import make_identity
from concourse.library_config import mlp as mlp_lib

BF16 = ml_dtypes.bfloat16

N = 100000
E = 3200000
NC = 8
S = 12500            # nodes per core
NT = 98              # node tiles per core
SP = NT * 128        # 12544 padded nodes per core
QR = 2 * SP          # table rows per quarter (2 core shards)
TBL = NC * SP        # all-gathered table rows
BLK = 1024           # edges per gather call (SWDGE ucode limit)
CHK = 128            # edges per selector chunk
CPC = 16             # gather calls per critical section
NBUF = 32            # rotating msg buffers (>= 2*CPC)
NS = 8               # rotating gather-dma semaphores
FIN, HID, CLS, K = 512, 16, 40, 2

_cache = {}


def _wrap16(idx):
    """[n] int -> [128, n//16] int16: pos i at [i%16, i//16], replicated x8."""
    n = idx.shape[0]
    w = idx.astype(np.int16).reshape(n // 16, 16).T
    return np.ascontiguousarray(np.tile(w, (8, 1)))


def _preprocess(x, edge_index, iw1, rw1, b1, iw2, rw2, b2):
    src = edge_index[0].astype(np.int64)
    dst = edge_index[1].astype(np.int64)
    deg = np.bincount(dst, minlength=N).astype(np.float32)
    dinv = np.where(deg > 0, 1.0 / np.sqrt(deg), 0.0).astype(np.float32)

    core = dst // S
    q = src // (2 * S)                      # source quarter
    trow = (src // S) * SP + (src % S) - QR * q   # quarter-relative table row
    dl = dst - core * S                     # local dst row [0,12500)
    tile = dl // 128
    dloc = dl % 128

    # global (core-uniform) chunk schedule: chunks per (q, tile)
    cnt = np.zeros((NC, 4, NT), np.int64)
    np.add.at(cnt, (core, q, tile), 1)
    kqt = np.ceil(cnt.max(axis=0) / CHK).astype(np.int64)      # [4, NT]
    kqt = np.maximum(kqt, 3)   # >=3 also guards the ds-wait/ceil4 deadlock
    nchunks_q = kqt.sum(axis=1)                                # per quarter
    # call layout per quarter: calls of <=64 chunks
    calls = []                       # list of (q, chunk_off_in_stream, nchunks)
    chunk_base_q = np.zeros(4, np.int64)
    off = 0
    for qq in range(4):
        chunk_base_q[qq] = off
        rem = int(nchunks_q[qq])
        pos = off
        while rem > 0:
            take = min(BLK // CHK, rem)
            calls.append((qq, pos, take))
            pos += take
            rem -= take
        off += int(nchunks_q[qq])
    nchunks = int(off)
    strm = nchunks * CHK

    # chunk start position of each (q, tile)
    tile_chunk_base = np.zeros((4, NT), np.int64)
    for qq in range(4):
        tile_chunk_base[qq] = chunk_base_q[qq] + np.r_[0, np.cumsum(kqt[qq])[:-1]]

    # schedule rows for _build: per chunk -> (q, tile, first, last)
    sched = []
    for qq in range(4):
        for t in range(NT):
            kk = int(kqt[qq, t])
            for j in range(kk):
                sched.append((qq, t, j == 0, j == kk - 1))

    # per-core stream fill
    # ascending src rows inside each (q,tile) group: the SWDGE gather
    # then reads DRAM in ascending order (row-buffer locality); the dloc
    # stream carries the slot mapping so consumption is unaffected.
    order = np.lexsort((trow, tile, q, core))
    src_s = trow[order]
    q_s, t_s, dloc_s = q[order], tile[order], dloc[order]
    core_s = core[order]

    gidx_all, dloc_all = [], []
    for c in range(NC):
        m = core_s == c
        gq, gt, gd, gs = q_s[m], t_s[m], dloc_s[m], src_s[m]
        # rank within (q, tile)
        key = gq * NT + gt
        # edges are sorted by (q, tile, dloc); rank = arange - group start
        starts = np.r_[0, np.flatnonzero(np.diff(key)) + 1]
        grp_start = np.zeros(len(key), np.int64)
        grp_start[starts] = starts
        grp_start = np.maximum.accumulate(grp_start)
        rank = np.arange(len(key)) - grp_start
        pos = tile_chunk_base[gq, gt] * CHK + rank
        gidx = np.zeros(strm, np.int64)
        dlv = np.full(strm, -1.0, np.float32)
        gidx[pos] = gs
        dlv[pos] = gd
        gidx_all.append(_wrap16(gidx))
        dloc_all.append(
            np.ascontiguousarray(
                dlv.reshape(nchunks, CHK).T.astype(BF16)))   # [128, nchunks]

    # weights
    iwcat1 = np.concatenate([iw1[0], iw1[1]], axis=1)        # [512, 32]
    rwcat1 = np.concatenate([rw1[0], rw1[1]], axis=1)        # [512, 32]
    w1 = np.ascontiguousarray(
        np.concatenate([iwcat1, rwcat1], axis=1)).astype(BF16)  # [512, 64]
    b1r = np.tile(np.concatenate([b1[0, 0], b1[1, 0]])[None, :], (128, 1)).astype(np.float32)
    w2 = np.zeros((32, 80), np.float32)
    for k in range(K):
        w2[0:16, 40 * k:40 * k + 40] = iw2[k]
        w2[16:32, 40 * k:40 * k + 40] = rw2[k]
    b2r = np.tile(np.concatenate([b2[0, 0], b2[1, 0]])[None, :], (128, 1)).astype(np.float32)

    iota = np.tile(np.arange(128, dtype=np.float32)[None, :], (128, 1)).astype(BF16)

    in_maps = []
    for c in range(NC):
        xT = np.zeros((FIN, SP), np.float32)
        xT[:, :S] = x[c * S:(c + 1) * S].T
        dv = np.zeros((128, NT), np.float32)
        dvp = np.zeros(SP, np.float32)
        dvp[:S] = dinv[c * S:(c + 1) * S]
        dv[:, :] = dvp.reshape(NT, 128).T
        in_maps.append({
            "xT": np.ascontiguousarray(xT.astype(BF16)),
            "gidx": gidx_all[c],
            "dloc": dloc_all[c],
            "iota": iota,
            "dinv_t": dv,
            "w1": w1,
            "b1r": b1r,
            "w2": w2,
            "b2r": b2r,
        })
    meta = {"calls": calls, "sched": sched, "nchunks": nchunks, "strm": strm}
    return in_maps, meta


def _build(meta):
    calls = meta["calls"]
    sched = meta["sched"]
    nchunks = meta["nchunks"]
    strm = meta["strm"]

    nc = bacc.Bacc("TRN2", target_bir_lowering=False, num_devices=NC)
    dt = mybir.dt
    f32 = dt.float32
    bf16 = dt.bfloat16

    xT_p = nc.declare_dram_parameter("xT", [FIN, SP], bf16, isOutput=False)
    gidx_p = nc.declare_dram_parameter("gidx", [128, strm // 16], dt.int16, isOutput=False)
    dloc_p = nc.declare_dram_parameter("dloc", [128, nchunks], bf16, isOutput=False)
    iota_p = nc.declare_dram_parameter("iota", [128, 128], bf16, isOutput=False)
    dinv_p = nc.declare_dram_parameter("dinv_t", [128, NT], f32, isOutput=False)
    w1_p = nc.declare_dram_parameter("w1", [FIN, 64], bf16, isOutput=False)
    b1r_p = nc.declare_dram_parameter("b1r", [128, 32], f32, isOutput=False)
    w2_p = nc.declare_dram_parameter("w2", [32, 80], f32, isOutput=False)
    b2r_p = nc.declare_dram_parameter("b2r", [128, 80], f32, isOutput=False)
    out_p = nc.declare_dram_parameter("out", [S, CLS], f32, isOutput=True)

    ag1_in = nc.dram_tensor("ag1_in", [SP, 128], bf16)
    ag1_out = nc.dram_tensor("ag1_out", [TBL, 128], bf16, addr_space="Shared")
    ag2_in = nc.dram_tensor("ag2_in", [SP, 128], bf16)
    ag2_out = nc.dram_tensor("ag2_out", [TBL, 128], bf16, addr_space="Shared")

    gs = [nc.alloc_semaphore(f"gs{i}") for i in range(NS)]
    cc_sem = nc.alloc_semaphore("cc_sem")
    rg = [list(range(NC))]

    with TileContext(nc) as tc:
        with (
            tc.tile_pool(name="const", bufs=1) as cp,
            tc.tile_pool(name="work", bufs=3) as wp,
            tc.tile_pool(name="edge", bufs=1) as ep,
            tc.tile_pool(name="psum", bufs=2, space="PSUM") as pp,
            tc.tile_pool(name="psum1", bufs=2, space="PSUM") as pp1,
            tc.tile_pool(name="selp", bufs=8) as sp,
        ):
            # ---- resident tiles ----
            gidx_sb = cp.tile([128, strm // 16], dt.int16)
            nc.sync.dma_start(gidx_sb[:], gidx_p[:])
            dloc_sb = cp.tile([128, nchunks], bf16)
            nc.sync.dma_start(dloc_sb[:], dloc_p[:])
            iota_sb = cp.tile([128, 128], bf16)
            nc.sync.dma_start(iota_sb[:], iota_p[:])
            dinv_sb = cp.tile([128, NT], f32)
            nc.sync.dma_start(dinv_sb[:], dinv_p[:])
            w1_sb = cp.tile([128, 4, 64], bf16)
            nc.sync.dma_start(w1_sb[:], w1_p[:].rearrange("(k p) n -> p k n", p=128))
            b1r_sb = cp.tile([128, 32], f32)
            nc.sync.dma_start(b1r_sb[:], b1r_p[:])
            w2_sb = cp.tile([32, 80], f32)
            nc.sync.dma_start(w2_sb[:], w2_p[:])
            b2r_sb = cp.tile([128, 80], f32)
            nc.sync.dma_start(b2r_sb[:], b2r_p[:])
            ident = cp.tile([128, 128], f32)
            make_identity(nc, ident[:])
            r1_res = cp.tile([128, NT * 32], f32)
            h_res = cp.tile([128, NT * 16], f32)
            agg1 = cp.tile([128, NT, 32], f32)
            agg2 = cp.tile([128, NT, 16], f32)
            scratch = cp.tile([128, 32], bf16)

            msgs = [ep.tile([128, BLK // CHK, 128], bf16, name=f"msg{i}")
                    for i in range(NBUF)]

            # ---- stage 1: projections x @ [iwcat|rwcat] -> L1 table ----
            for t in range(NT):
                xt = wp.tile([128, 4, 128], bf16, tag="xt")
                nc.sync.dma_start(
                    xt[:], xT_p[:, t * 128:(t + 1) * 128].rearrange("(k p) m -> p k m", p=128))
                hps = pp.tile([128, 64], f32, space="PSUM", tag="hps")
                for k in range(4):
                    nc.tensor.matmul(hps[:], lhsT=xt[:, k, :], rhs=w1_sb[:, k, :],
                                     start=(k == 0), stop=(k == 3))
                h1s = wp.tile([128, 32], bf16, tag="h1s")
                nc.vector.tensor_tensor(
                    out=h1s[:], in0=hps[:, 0:32],
                    in1=dinv_sb[:, t:t + 1].to_broadcast([128, 32]),
                    op=mybir.AluOpType.mult)
                nc.sync.dma_start(ag1_in[t * 128:(t + 1) * 128, 0:32], h1s[:])
                nc.scalar.copy(r1_res[:, 32 * t:32 * t + 32], hps[:, 32:64])

            # ---- stage 2: AllGather L1 table ----
            nc.gpsimd.collective_compute(
                "AllGather", mybir.AluOpType.bypass, replica_groups=rg,
                ins=[ag1_in[:].opt()], outs=[ag1_out[:].opt()])

            # ---- edge phase (shared for both layers) ----
            # chunk ci -> call index
            chunk_call = np.zeros(nchunks, np.int64)
            for j, (qq, coff, nch) in enumerate(calls):
                chunk_call[coff:coff + nch] = j

            # counters persist across the two passes
            cnts = {"g": [0] * NS, "pass": 0}

            def edge_pass(table, agg, fw, epi_cb=None):
                """fw: feature width of rhs slice (32 for L1, 16 for L2).
                epi_cb(t) is emitted right after tile t's final drain so its
                compute overlaps the remaining edge-phase tail."""
                cnts["pass"] += 1
                pn = cnts["pass"]
                ncalls = len(calls)
                gthr = {}

                def finish_group(lo, hi):
                    """Wait for group [lo,hi)'s gather DMAs (keeps desc-gen
                    paced against the ring as in v2)."""
                    for j in range(lo, hi):
                        nc.gpsimd.wait_ge(gs[j % NS], 16 * gthr[j])

                def mark_group(lo, hi):
                    """Re-mark landed buffers (tiny write into the unread
                    junk column zone) so Tile orders consumers after the data
                    actually landed -- on the idle Scalar engine so the
                    887 markers stop serializing the gpsimd queue."""
                    with tc.tile_critical():
                        for j in range(lo, hi):
                            nc.scalar.wait_ge(gs[j % NS], 16 * gthr[j])
                            nc.scalar.memzero(msgs[j % NBUF][0:1, 0:1, 64:66])

                def gather_group(cg):
                    """One critical: issue gathers for calls cg..cg+CPC-1;
                    completion of the PREVIOUS group is waited here, so its
                    DMA tail hides behind this group's descriptor generation."""
                    lo, hi = cg, min(cg + CPC, ncalls)
                    with tc.tile_critical():
                        if cg == 0:
                            # bind collective completion (custom DMA can't
                            # carry walrus waits): probe-read the AG output.
                            nc.gpsimd.memset(scratch[:], 0.0)
                            nc.gpsimd.dma_start(
                                scratch[0:1, 0:32],
                                table[0:1, 0:32]).then_inc(cc_sem, 16)
                            nc.gpsimd.wait_ge(cc_sem, 16 * pn)
                            nc.gpsimd.load_library(mlp_lib)
                        for j in range(lo, hi):
                            qq, coff, nch = calls[j]
                            eoff = coff * CHK
                            nidx = nch * CHK
                            cnts["g"][j % NS] += 1
                            gthr[j] = cnts["g"][j % NS]
                            nc.gpsimd.dma_gather(
                                out_ap=msgs[j % NBUF][:, 0:nch, :],
                                in_ap=table[QR * qq:QR * (qq + 1), :],
                                idxs_ap=gidx_sb[:, eoff // 16:(eoff + nidx) // 16],
                                num_idxs=nidx, num_idxs_reg=nidx, elem_size=128,
                            ).then_inc(gs[j % NS], 16)
                        if cg > 0:
                            finish_group(cg - CPC, cg)
                    if cg > 0:
                        mark_group(cg - CPC, cg)

                def tail_group(cg):
                    with tc.tile_critical():
                        finish_group(cg, ncalls)
                    mark_group(cg, ncalls)

                # Tile-managed consumption; gather criticals interleave so
                # the scheduler can overlap desc-gen with PE/DVE consumption.
                issued = 0
                pst = None
                last_lo = ((ncalls - 1) // CPC) * CPC
                tail_done = False
                for ci, (qq, t, first, last) in enumerate(sched):
                    j = int(chunk_call[ci])
                    # issue gather groups ahead of consumption; the bound
                    # guarantees no buffer in the group can overwrite data
                    # whose reader instructions aren't emitted yet.
                    while issued < ncalls and issued <= j + NBUF - CPC:
                        gather_group(issued)
                        issued += CPC
                    if j >= last_lo and not tail_done:
                        tail_done = True
                        tail_group(last_lo)
                    sel = sp.tile([128, 128], bf16, tag="sel")
                    nc.vector.tensor_tensor(
                        out=sel[:], in0=iota_sb[:],
                        in1=dloc_sb[:, ci:ci + 1].to_broadcast([128, 128]),
                        op=mybir.AluOpType.is_equal)
                    if first:
                        pst = pp1.tile([128, fw], f32, space="PSUM", tag="eps")
                    cin = ci - calls[j][1]
                    nc.tensor.matmul(
                        pst[:], lhsT=sel[:],
                        rhs=msgs[j % NBUF][:, cin, 0:fw],
                        start=first, stop=last)
                    if last:
                        sl = agg[:, t, :]
                        if qq == 0:
                            nc.vector.tensor_scalar(
                                out=sl, in0=pst[:], scalar1=1.0,
                                scalar2=None, op0=mybir.AluOpType.mult)
                        else:
                            nc.vector.tensor_tensor(
                                out=sl, in0=pst[:], in1=sl,
                                op=mybir.AluOpType.add)
                        if qq == 3 and epi_cb is not None:
                            epi_cb(t)

            def epi1(t):
                dvb = dinv_sb[:, t:t + 1].to_broadcast([128, 32])
                v = wp.tile([128, 32], f32, tag="v")
                nc.vector.tensor_tensor(out=v[:], in0=agg1[:, t, :], in1=dvb,
                                        op=mybir.AluOpType.mult)
                nc.vector.tensor_tensor(out=v[:], in0=v[:], in1=r1_res[:, 32 * t:32 * t + 32],
                                        op=mybir.AluOpType.add)
                nc.vector.tensor_tensor(out=v[:], in0=v[:], in1=b1r_sb[:],
                                        op=mybir.AluOpType.add)
                nc.vector.tensor_scalar(out=v[:], in0=v[:], scalar1=0.0, scalar2=None,
                                        op0=mybir.AluOpType.max)
                h = h_res[:, 16 * t:16 * t + 16]
                nc.vector.tensor_tensor(out=h, in0=v[:, 0:16], in1=v[:, 16:32],
                                        op=mybir.AluOpType.add)
                nc.vector.tensor_scalar(out=h, in0=h, scalar1=0.5, scalar2=None,
                                        op0=mybir.AluOpType.mult)
                h2s = wp.tile([128, 16], bf16, tag="h2s")
                nc.vector.tensor_tensor(
                    out=h2s[:], in0=h,
                    in1=dinv_sb[:, t:t + 1].to_broadcast([128, 16]),
                    op=mybir.AluOpType.mult)
                nc.sync.dma_start(ag2_in[t * 128:(t + 1) * 128, 0:16], h2s[:])

            edge_pass(ag1_out, agg1, 32, epi_cb=epi1)

            # ---- AllGather L2 table ----
            nc.gpsimd.collective_compute(
                "AllGather", mybir.AluOpType.bypass, replica_groups=rg,
                ins=[ag2_in[:].opt()], outs=[ag2_out[:].opt()])

            # ---- L2 epilogue (per tile, overlapped via epi_cb) ----
            def epi2(t):
                cc = wp.tile([128, 32], f32, tag="cc")
                nc.vector.tensor_tensor(
                    out=cc[:, 0:16], in0=agg2[:, t, :],
                    in1=dinv_sb[:, t:t + 1].to_broadcast([128, 16]),
                    op=mybir.AluOpType.mult)
                nc.scalar.copy(cc[:, 16:32], h_res[:, 16 * t:16 * t + 16])
                ccT_ps = pp.tile([32, 128], f32, space="PSUM", tag="ccT")
                nc.tensor.transpose(out=ccT_ps[:], in_=cc[:], identity=ident[:])
                ccT = wp.tile([32, 128], f32, tag="ccTs")
                nc.scalar.copy(ccT[:], ccT_ps[:])
                ops = pp.tile([128, 80], f32, space="PSUM", tag="ops")
                nc.tensor.matmul(ops[:], lhsT=ccT[:], rhs=w2_sb[:], start=True, stop=True)
                o = wp.tile([128, 80], f32, tag="o")
                nc.vector.tensor_tensor(out=o[:], in0=ops[:], in1=b2r_sb[:],
                                        op=mybir.AluOpType.add)
                nc.vector.tensor_scalar(out=o[:], in0=o[:], scalar1=0.0, scalar2=None,
                                        op0=mybir.AluOpType.max)
                fin = wp.tile([128, CLS], f32, tag="fin")
                nc.vector.tensor_tensor(out=fin[:], in0=o[:, 0:40], in1=o[:, 40:80],
                                        op=mybir.AluOpType.add)
                nc.vector.tensor_scalar(out=fin[:], in0=fin[:], scalar1=0.5, scalar2=None,
                                        op0=mybir.AluOpType.mult)
                lo = t * 128
                hi = min(lo + 128, S)
                if hi > lo:
                    nc.sync.dma_start(out_p[lo:hi, :], fin[0:hi - lo, :])

            edge_pass(ag2_out, agg2, 16, epi_cb=epi2)

    nc.compile()
    return nc


def _make_runner(nc, n_cores=NC):
    import jax
    from jax.sharding import Mesh, PartitionSpec, NamedSharding
    from jax.experimental.shard_map import shard_map
    from concourse.bass2jax import (
        _bass_exec_p, install_neuronx_cc_hook, partition_id_tensor)

    install_neuronx_cc_hook()
    partition_name = nc.partition_id_tensor.name if nc.partition_id_tensor else None
    in_names, out_names, out_avals, zero_outs = [], [], [], []
    for alloc in nc.m.functions[0].allocations:
        if not isinstance(alloc, mybir.MemoryLocationSet):
            continue
        name = alloc.memorylocations[0].name
        if alloc.kind == "ExternalInput":
            if name != partition_name:
                in_names.append(name)
        elif alloc.kind == "ExternalOutput":
            out_names.append(name)
            shape = tuple(alloc.tensor_shape)
            dtype = mybir.dt.np(alloc.dtype)
            out_avals.append(jax.core.ShapedArray(shape, dtype))
            zero_outs.append(np.zeros(shape, dtype))
    n_params = len(in_names)
    in_names_full = list(in_names) + out_names
    if partition_name is not None:
        in_names_full.append(partition_name)

    def _body(*args):
        operands = list(args)
        if partition_name is not None:
            operands.append(partition_id_tensor())
        outs = _bass_exec_p.bind(
            *operands,
            out_avals=tuple(out_avals),
            in_names=tuple(in_names_full),
            out_names=tuple(out_names),
            lowering_input_output_aliases=(),
            sim_require_finite=True,
            sim_require_nnan=True,
            nc=nc,
        )
        return tuple(outs)

    devices = jax.devices()[:n_cores]
    mesh = Mesh(np.asarray(devices), ("core",))
    in_specs = (PartitionSpec("core"),) * (n_params + len(out_names))
    out_specs = (PartitionSpec("core"),) * len(out_names)
    sharded = jax.jit(
        shard_map(_body, mesh=mesh, in_specs=in_specs, out_specs=out_specs,
                  check_rep=False),
        keep_unused=True)

    def run(in_maps, repeats=1):
        sh = NamedSharding(mesh, PartitionSpec("core"))
        per_core = [[np.asarray(m[k]) for k in in_names] for m in in_maps]
        concat_in = [
            jax.device_put(
                np.concatenate([per_core[c][i] for c in range(n_cores)], axis=0), sh)
            for i in range(n_params)
        ]
        concat_zeros = [
            jax.device_put(
                np.zeros((n_cores * z.shape[0], *z.shape[1:]), z.dtype), sh)
            for z in zero_outs
        ]
        import jax as _j
        _j.block_until_ready(concat_in)
        _j.block_until_ready(concat_zeros)
        times = []
        out_arrs = None
        for _ in range(repeats):
            t0 = time.perf_counter()
            out_arrs = sharded(*concat_in, *concat_zeros)
            _j.block_until_ready(out_arrs)
            times.append(time.perf_counter() - t0)
        results = [
            {name: np.asarray(out_arrs[i]).reshape(n_cores, *out_avals[i].shape)[c]
             for i, name in enumerate(out_names)}
            for c in range(n_cores)
        ]
        return results, times

    return run



def kernel(x, edge_index, iw1, rw1, b1, iw2, rw2, b2, _timing=None, _expose=None):
    x = np.asarray(x, dtype=np.float32)
    edge_index = np.asarray(edge_index)
    in_maps, meta = _preprocess(
        x, edge_index, np.asarray(iw1), np.asarray(rw1), np.asarray(b1),
        np.asarray(iw2), np.asarray(rw2), np.asarray(b2))
    key = ("v6", meta["nchunks"], meta["strm"])
    if key not in _cache:
        nc = _build(meta)
        _cache[key] = (_make_runner(nc), nc)
    run, nc = _cache[key]
    repeats = 30 if _timing is not None else 1
    results, times = run(in_maps, repeats=repeats)
    if _timing is not None:
        _timing.extend(times)
    if _expose is not None:
        _expose.update({"run": run, "in_maps": in_maps, "nc": nc})
    out = np.concatenate([results[c]["out"] for c in range(NC)], axis=0)
    return out



# revision 16
# speedup vs baseline: 2.7064x; 1.1606x over previous
"""ARMA GNN (2-layer, K=2 stacks) distributed Bass kernel for 8 TRN2 NeuronCores.

v2: dst-sorted edge streams + PE selector-matmul segment reduction.
 - Nodes sharded 12500/core; edges partitioned by destination core.
 - Layer math refactored so message passing happens at small feature dims
   (32 cols L1, 16 cols L2), with the stack projections applied before (L1)
   or after (L2) aggregation.
 - The all-gathered per-node table is stored bf16, one node per 256B row.
 - Edge phase per layer: SWDGE dma_gather fetches src rows in dst-sorted
   order (8192 edges/call); for every 128-edge chunk the vector engine
   builds a one-hot selector S^T[j, i] = (dloc[j] == i) and the tensor
   engine accumulates S^T^T @ msg into the dst tile's PSUM bank. No
   scatter descriptors, no conflict packing; f32 PSUM accumulation.
 - agg lives in SBUF; epilogues identical in spirit to v1.
"""
import sys
import time

sys.path.insert(0, "/opt/trn_rl_repo")

import numpy as np
import ml_dtypes

import concourse.bass as bass
import concourse.bacc as bacc
import concourse.mybir as mybir
from concourse.tile import TileContext
from concourse.masks import make_identity
from concourse.library_config import mlp as mlp_lib

BF16 = ml_dtypes.bfloat16

N = 100000
E = 3200000
NC = 8
S = 12500            # nodes per core
NT = 98              # node tiles per core
SP = NT * 128        # 12544 padded nodes per core
QR = 2 * SP          # table rows per quarter (2 core shards)
TBL = NC * SP        # all-gathered table rows
BLK = 1024           # edges per gather call (SWDGE ucode limit)
CHK = 128            # edges per selector chunk
CPC = 16             # gather calls per critical section
NBUF = 32            # rotating msg buffers (>= 2*CPC)
NS = 8               # rotating gather-dma semaphores
FIN, HID, CLS, K = 512, 16, 40, 2

_cache = {}


def _wrap16(idx):
    """[n] int -> [128, n//16] int16: pos i at [i%16, i//16], replicated x8."""
    n = idx.shape[0]
    w = idx.astype(np.int16).reshape(n // 16, 16).T
    return np.ascontiguousarray(np.tile(w, (8, 1)))


def _preprocess(x, edge_index, iw1, rw1, b1, iw2, rw2, b2):
    src = edge_index[0].astype(np.int64)
    dst = edge_index[1].astype(np.int64)
    deg = np.bincount(dst, minlength=N).astype(np.float32)
    dinv = np.where(deg > 0, 1.0 / np.sqrt(deg), 0.0).astype(np.float32)

    core = dst // S
    q = src // (2 * S)                      # source quarter
    trow = (src // S) * SP + (src % S) - QR * q   # quarter-relative table row
    dl = dst - core * S                     # local dst row [0,12500)
    tile = dl // 128
    dloc = dl % 128

    # global (core-uniform) chunk schedule: chunks per (q, tile)
    cnt = np.zeros((NC, 4, NT), np.int64)
    np.add.at(cnt, (core, q, tile), 1)
    kqt = np.ceil(cnt.max(axis=0) / CHK).astype(np.int64)      # [4, NT]
    kqt = np.maximum(kqt, 3)   # >=3 also guards the ds-wait/ceil4 deadlock
    nchunks_q = kqt.sum(axis=1)                                # per quarter
    # call layout per quarter: calls of <=64 chunks
    calls = []                       # list of (q, chunk_off_in_stream, nchunks)
    chunk_base_q = np.zeros(4, np.int64)
    off = 0
    for qq in range(4):
        chunk_base_q[qq] = off
        rem = int(nchunks_q[qq])
        pos = off
        while rem > 0:
            take = min(BLK // CHK, rem)
            calls.append((qq, pos, take))
            pos += take
            rem -= take
        off += int(nchunks_q[qq])
    nchunks = int(off)
    strm = nchunks * CHK

    # chunk start position of each (q, tile)
    tile_chunk_base = np.zeros((4, NT), np.int64)
    for qq in range(4):
        tile_chunk_base[qq] = chunk_base_q[qq] + np.r_[0, np.cumsum(kqt[qq])[:-1]]

    # schedule rows for _build: per chunk -> (q, tile, first, last)
    sched = []
    for qq in range(4):
        for t in range(NT):
            kk = int(kqt[qq, t])
            for j in range(kk):
                sched.append((qq, t, j == 0, j == kk - 1))

    # per-core stream fill
    # ascending src rows inside each (q,tile) group: the SWDGE gather
    # then reads DRAM in ascending order (row-buffer locality); the dloc
    # stream carries the slot mapping so consumption is unaffected.
    order = np.lexsort((trow, tile, q, core))
    src_s = trow[order]
    q_s, t_s, dloc_s = q[order], tile[order], dloc[order]
    core_s = core[order]

    gidx_all, dloc_all = [], []
    for c in range(NC):
        m = core_s == c
        gq, gt, gd, gs = q_s[m], t_s[m], dloc_s[m], src_s[m]
        # rank within (q, tile)
        key = gq * NT + gt
        # edges are sorted by (q, tile, dloc); rank = arange - group start
        starts = np.r_[0, np.flatnonzero(np.diff(key)) + 1]
        grp_start = np.zeros(len(key), np.int64)
        grp_start[starts] = starts
        grp_start = np.maximum.accumulate(grp_start)
        rank = np.arange(len(key)) - grp_start
        pos = tile_chunk_base[gq, gt] * CHK + rank
        gidx = np.zeros(strm, np.int64)
        dlv = np.full(strm, -1.0, np.float32)
        gidx[pos] = gs
        dlv[pos] = gd
        gidx_all.append(_wrap16(gidx))
        dloc_all.append(
            np.ascontiguousarray(
                dlv.reshape(nchunks, CHK).T.astype(BF16)))   # [128, nchunks]

    # weights
    iwcat1 = np.concatenate([iw1[0], iw1[1]], axis=1)        # [512, 32]
    rwcat1 = np.concatenate([rw1[0], rw1[1]], axis=1)        # [512, 32]
    w1 = np.ascontiguousarray(
        np.concatenate([iwcat1, rwcat1], axis=1)).astype(BF16)  # [512, 64]
    b1r = np.tile(np.concatenate([b1[0, 0], b1[1, 0]])[None, :], (128, 1)).astype(np.float32)
    w2 = np.zeros((32, 80), np.float32)
    for k in range(K):
        w2[0:16, 40 * k:40 * k + 40] = iw2[k]
        w2[16:32, 40 * k:40 * k + 40] = rw2[k]
    b2r = np.tile(np.concatenate([b2[0, 0], b2[1, 0]])[None, :], (128, 1)).astype(np.float32)

    iota = np.tile(np.arange(128, dtype=np.float32)[None, :], (128, 1)).astype(BF16)

    in_maps = []
    for c in range(NC):
        xT = np.zeros((FIN, SP), np.float32)
        xT[:, :S] = x[c * S:(c + 1) * S].T
        dv = np.zeros((128, NT), np.float32)
        dvp = np.zeros(SP, np.float32)
        dvp[:S] = dinv[c * S:(c + 1) * S]
        dv[:, :] = dvp.reshape(NT, 128).T
        in_maps.append({
            "xT": np.ascontiguousarray(xT.astype(BF16)),
            "gidx": gidx_all[c],
            "dloc": dloc_all[c],
            "iota": iota,
            "dinv_t": dv,
            "w1": w1,
            "b1r": b1r,
            "w2": w2,
            "b2r": b2r,
        })
    meta = {"calls": calls, "sched": sched, "nchunks": nchunks, "strm": strm}
    return in_maps, meta


def _build(meta):
    calls = meta["calls"]
    sched = meta["sched"]
    nchunks = meta["nchunks"]
    strm = meta["strm"]

    nc = bacc.Bacc("TRN2", target_bir_lowering=False, num_devices=NC,
               num_swdge_queues=2)
    dt = mybir.dt
    f32 = dt.float32
    bf16 = dt.bfloat16

    xT_p = nc.declare_dram_parameter("xT", [FIN, SP], bf16, isOutput=False)
    gidx_p = nc.declare_dram_parameter("gidx", [128, strm // 16], dt.int16, isOutput=False)
    dloc_p = nc.declare_dram_parameter("dloc", [128, nchunks], bf16, isOutput=False)
    iota_p = nc.declare_dram_parameter("iota", [128, 128], bf16, isOutput=False)
    dinv_p = nc.declare_dram_parameter("dinv_t", [128, NT], f32, isOutput=False)
    w1_p = nc.declare_dram_parameter("w1", [FIN, 64], bf16, isOutput=False)
    b1r_p = nc.declare_dram_parameter("b1r", [128, 32], f32, isOutput=False)
    w2_p = nc.declare_dram_parameter("w2", [32, 80], f32, isOutput=False)
    b2r_p = nc.declare_dram_parameter("b2r", [128, 80], f32, isOutput=False)
    out_p = nc.declare_dram_parameter("out", [S, CLS], f32, isOutput=True)

    ag1_in = nc.dram_tensor("ag1_in", [SP, 128], bf16)
    ag1_out = nc.dram_tensor("ag1_out", [TBL, 128], bf16, addr_space="Shared")
    ag2_in = nc.dram_tensor("ag2_in", [SP, 128], bf16)
    ag2_out = nc.dram_tensor("ag2_out", [TBL, 128], bf16, addr_space="Shared")

    gs = [nc.alloc_semaphore(f"gs{i}") for i in range(NS)]
    cc_sem = nc.alloc_semaphore("cc_sem")
    rg = [list(range(NC))]

    with TileContext(nc) as tc:
        with (
            tc.tile_pool(name="const", bufs=1) as cp,
            tc.tile_pool(name="work", bufs=3) as wp,
            tc.tile_pool(name="edge", bufs=1) as ep,
            tc.tile_pool(name="psum", bufs=2, space="PSUM") as pp,
            tc.tile_pool(name="psum1", bufs=2, space="PSUM") as pp1,
            tc.tile_pool(name="selp", bufs=8) as sp,
        ):
            # ---- resident tiles ----
            gidx_sb = cp.tile([128, strm // 16], dt.int16)
            nc.sync.dma_start(gidx_sb[:], gidx_p[:])
            dloc_sb = cp.tile([128, nchunks], bf16)
            nc.sync.dma_start(dloc_sb[:], dloc_p[:])
            iota_sb = cp.tile([128, 128], bf16)
            nc.sync.dma_start(iota_sb[:], iota_p[:])
            dinv_sb = cp.tile([128, NT], f32)
            nc.sync.dma_start(dinv_sb[:], dinv_p[:])
            w1_sb = cp.tile([128, 4, 64], bf16)
            nc.sync.dma_start(w1_sb[:], w1_p[:].rearrange("(k p) n -> p k n", p=128))
            b1r_sb = cp.tile([128, 32], f32)
            nc.sync.dma_start(b1r_sb[:], b1r_p[:])
            w2_sb = cp.tile([32, 80], f32)
            nc.sync.dma_start(w2_sb[:], w2_p[:])
            b2r_sb = cp.tile([128, 80], f32)
            nc.sync.dma_start(b2r_sb[:], b2r_p[:])
            ident = cp.tile([128, 128], f32)
            make_identity(nc, ident[:])
            r1_res = cp.tile([128, NT * 32], f32)
            h_res = cp.tile([128, NT * 16], f32)
            agg1 = cp.tile([128, NT, 32], f32)
            agg2 = cp.tile([128, NT, 16], f32)
            scratch = cp.tile([128, 32], bf16)

            msgs = [ep.tile([128, BLK // CHK, 128], bf16, name=f"msg{i}")
                    for i in range(NBUF)]

            # ---- stage 1: projections x @ [iwcat|rwcat] -> L1 table ----
            for t in range(NT):
                xt = wp.tile([128, 4, 128], bf16, tag="xt")
                nc.sync.dma_start(
                    xt[:], xT_p[:, t * 128:(t + 1) * 128].rearrange("(k p) m -> p k m", p=128))
                hps = pp.tile([128, 64], f32, space="PSUM", tag="hps")
                for k in range(4):
                    nc.tensor.matmul(hps[:], lhsT=xt[:, k, :], rhs=w1_sb[:, k, :],
                                     start=(k == 0), stop=(k == 3))
                h1s = wp.tile([128, 32], bf16, tag="h1s")
                nc.vector.tensor_tensor(
                    out=h1s[:], in0=hps[:, 0:32],
                    in1=dinv_sb[:, t:t + 1].to_broadcast([128, 32]),
                    op=mybir.AluOpType.mult)
                nc.sync.dma_start(ag1_in[t * 128:(t + 1) * 128, 0:32], h1s[:])
                nc.scalar.copy(r1_res[:, 32 * t:32 * t + 32], hps[:, 32:64])

            # ---- stage 2: AllGather L1 table ----
            nc.gpsimd.collective_compute(
                "AllGather", mybir.AluOpType.bypass, replica_groups=rg,
                ins=[ag1_in[:].opt()], outs=[ag1_out[:].opt()])

            # ---- edge phase (shared for both layers) ----
            # chunk ci -> call index
            chunk_call = np.zeros(nchunks, np.int64)
            for j, (qq, coff, nch) in enumerate(calls):
                chunk_call[coff:coff + nch] = j

            # counters persist across the two passes
            cnts = {"g": [0] * NS, "pass": 0}

            def edge_pass(table, agg, fw, epi_cb=None):
                """fw: feature width of rhs slice (32 for L1, 16 for L2).
                epi_cb(t) is emitted right after tile t's final drain so its
                compute overlaps the remaining edge-phase tail."""
                cnts["pass"] += 1
                pn = cnts["pass"]
                ncalls = len(calls)
                gthr = {}

                def finish_group(lo, hi):
                    """Wait for group [lo,hi)'s gather DMAs (keeps desc-gen
                    paced against the ring as in v2)."""
                    for j in range(lo, hi):
                        nc.gpsimd.wait_ge(gs[j % NS], 16 * gthr[j])

                def mark_group(lo, hi):
                    """Re-mark landed buffers (tiny write into the unread
                    junk column zone) so Tile orders consumers after the data
                    actually landed -- on the idle Scalar engine so the
                    887 markers stop serializing the gpsimd queue."""
                    with tc.tile_critical():
                        for j in range(lo, hi):
                            nc.scalar.wait_ge(gs[j % NS], 16 * gthr[j])
                            nc.scalar.memzero(msgs[j % NBUF][0:1, 0:1, 64:66])

                def gather_group(cg):
                    """One critical: issue gathers for calls cg..cg+CPC-1;
                    completion of the PREVIOUS group is waited here, so its
                    DMA tail hides behind this group's descriptor generation."""
                    lo, hi = cg, min(cg + CPC, ncalls)
                    with tc.tile_critical():
                        if cg == 0:
                            # bind collective completion (custom DMA can't
                            # carry walrus waits): probe-read the AG output.
                            nc.gpsimd.memset(scratch[:], 0.0)
                            nc.gpsimd.dma_start(
                                scratch[0:1, 0:32],
                                table[0:1, 0:32]).then_inc(cc_sem, 16)
                            nc.gpsimd.wait_ge(cc_sem, 16 * pn)
                            nc.gpsimd.load_library(mlp_lib)
                        for j in range(lo, hi):
                            qq, coff, nch = calls[j]
                            eoff = coff * CHK
                            nidx = nch * CHK
                            cnts["g"][j % NS] += 1
                            gthr[j] = cnts["g"][j % NS]
                            nc.gpsimd.dma_gather(
                                out_ap=msgs[j % NBUF][:, 0:nch, :],
                                in_ap=table[QR * qq:QR * (qq + 1), :],
                                idxs_ap=gidx_sb[:, eoff // 16:(eoff + nidx) // 16],
                                num_idxs=nidx, num_idxs_reg=nidx, elem_size=128,
                                queue_num=j % 2,
                            ).then_inc(gs[j % NS], 16)
                        if cg > 0:
                            finish_group(cg - CPC, cg)
                    if cg > 0:
                        mark_group(cg - CPC, cg)

                def tail_group(cg):
                    with tc.tile_critical():
                        finish_group(cg, ncalls)
                    mark_group(cg, ncalls)

                # Tile-managed consumption; gather criticals interleave so
                # the scheduler can overlap desc-gen with PE/DVE consumption.
                issued = 0
                pst = None
                last_lo = ((ncalls - 1) // CPC) * CPC
                tail_done = False
                for ci, (qq, t, first, last) in enumerate(sched):
                    j = int(chunk_call[ci])
                    # issue gather groups ahead of consumption; the bound
                    # guarantees no buffer in the group can overwrite data
                    # whose reader instructions aren't emitted yet.
                    while issued < ncalls and issued <= j + NBUF - CPC:
                        gather_group(issued)
                        issued += CPC
                    if j >= last_lo and not tail_done:
                        tail_done = True
                        tail_group(last_lo)
                    sel = sp.tile([128, 128], bf16, tag="sel")
                    nc.vector.tensor_tensor(
                        out=sel[:], in0=iota_sb[:],
                        in1=dloc_sb[:, ci:ci + 1].to_broadcast([128, 128]),
                        op=mybir.AluOpType.is_equal)
                    if first:
                        pst = pp1.tile([128, fw], f32, space="PSUM", tag="eps")
                    cin = ci - calls[j][1]
                    nc.tensor.matmul(
                        pst[:], lhsT=sel[:],
                        rhs=msgs[j % NBUF][:, cin, 0:fw],
                        start=first, stop=last)
                    if last:
                        sl = agg[:, t, :]
                        if qq == 0:
                            nc.vector.tensor_scalar(
                                out=sl, in0=pst[:], scalar1=1.0,
                                scalar2=None, op0=mybir.AluOpType.mult)
                        else:
                            nc.vector.tensor_tensor(
                                out=sl, in0=pst[:], in1=sl,
                                op=mybir.AluOpType.add)
                        if qq == 3 and epi_cb is not None:
                            epi_cb(t)

            def epi1(t):
                dvb = dinv_sb[:, t:t + 1].to_broadcast([128, 32])
                v = wp.tile([128, 32], f32, tag="v")
                nc.vector.tensor_tensor(out=v[:], in0=agg1[:, t, :], in1=dvb,
                                        op=mybir.AluOpType.mult)
                nc.vector.tensor_tensor(out=v[:], in0=v[:], in1=r1_res[:, 32 * t:32 * t + 32],
                                        op=mybir.AluOpType.add)
                nc.vector.tensor_tensor(out=v[:], in0=v[:], in1=b1r_sb[:],
                                        op=mybir.AluOpType.add)
                nc.vector.tensor_scalar(out=v[:], in0=v[:], scalar1=0.0, scalar2=None,
                                        op0=mybir.AluOpType.max)
                h = h_res[:, 16 * t:16 * t + 16]
                nc.vector.tensor_tensor(out=h, in0=v[:, 0:16], in1=v[:, 16:32],
                                        op=mybir.AluOpType.add)
                nc.vector.tensor_scalar(out=h, in0=h, scalar1=0.5, scalar2=None,
                                        op0=mybir.AluOpType.mult)
                h2s = wp.tile([128, 16], bf16, tag="h2s")
                nc.vector.tensor_tensor(
                    out=h2s[:], in0=h,
                    in1=dinv_sb[:, t:t + 1].to_broadcast([128, 16]),
                    op=mybir.AluOpType.mult)
                nc.sync.dma_start(ag2_in[t * 128:(t + 1) * 128, 0:16], h2s[:])

            edge_pass(ag1_out, agg1, 32, epi_cb=epi1)

            # ---- AllGather L2 table ----
            nc.gpsimd.collective_compute(
                "AllGather", mybir.AluOpType.bypass, replica_groups=rg,
                ins=[ag2_in[:].opt()], outs=[ag2_out[:].opt()])

            # ---- L2 epilogue (per tile, overlapped via epi_cb) ----
            def epi2(t):
                cc = wp.tile([128, 32], f32, tag="cc")
                nc.vector.tensor_tensor(
                    out=cc[:, 0:16], in0=agg2[:, t, :],
                    in1=dinv_sb[:, t:t + 1].to_broadcast([128, 16]),
                    op=mybir.AluOpType.mult)
                nc.scalar.copy(cc[:, 16:32], h_res[:, 16 * t:16 * t + 16])
                ccT_ps = pp.tile([32, 128], f32, space="PSUM", tag="ccT")
                nc.tensor.transpose(out=ccT_ps[:], in_=cc[:], identity=ident[:])
                ccT = wp.tile([32, 128], f32, tag="ccTs")
                nc.scalar.copy(ccT[:], ccT_ps[:])
                ops = pp.tile([128, 80], f32, space="PSUM", tag="ops")
                nc.tensor.matmul(ops[:], lhsT=ccT[:], rhs=w2_sb[:], start=True, stop=True)
                o = wp.tile([128, 80], f32, tag="o")
                nc.vector.tensor_tensor(out=o[:], in0=ops[:], in1=b2r_sb[:],
                                        op=mybir.AluOpType.add)
                nc.vector.tensor_scalar(out=o[:], in0=o[:], scalar1=0.0, scalar2=None,
                                        op0=mybir.AluOpType.max)
                fin = wp.tile([128, CLS], f32, tag="fin")
                nc.vector.tensor_tensor(out=fin[:], in0=o[:, 0:40], in1=o[:, 40:80],
                                        op=mybir.AluOpType.add)
                nc.vector.tensor_scalar(out=fin[:], in0=fin[:], scalar1=0.5, scalar2=None,
                                        op0=mybir.AluOpType.mult)
                lo = t * 128
                hi = min(lo + 128, S)
                if hi > lo:
                    nc.sync.dma_start(out_p[lo:hi, :], fin[0:hi - lo, :])

            edge_pass(ag2_out, agg2, 16, epi_cb=epi2)

    nc.compile()
    return nc


def _make_runner(nc, n_cores=NC):
    import jax
    from jax.sharding import Mesh, PartitionSpec, NamedSharding
    from jax.experimental.shard_map import shard_map
    from concourse.bass2jax import (
        _bass_exec_p, install_neuronx_cc_hook, partition_id_tensor)

    install_neuronx_cc_hook()
    partition_name = nc.partition_id_tensor.name if nc.partition_id_tensor else None
    in_names, out_names, out_avals, zero_outs = [], [], [], []
    for alloc in nc.m.functions[0].allocations:
        if not isinstance(alloc, mybir.MemoryLocationSet):
            continue
        name = alloc.memorylocations[0].name
        if alloc.kind == "ExternalInput":
            if name != partition_name:
                in_names.append(name)
        elif alloc.kind == "ExternalOutput":
            out_names.append(name)
            shape = tuple(alloc.tensor_shape)
            dtype = mybir.dt.np(alloc.dtype)
            out_avals.append(jax.core.ShapedArray(shape, dtype))
            zero_outs.append(np.zeros(shape, dtype))
    n_params = len(in_names)
    in_names_full = list(in_names) + out_names
    if partition_name is not None:
        in_names_full.append(partition_name)

    def _body(*args):
        operands = list(args)
        if partition_name is not None:
            operands.append(partition_id_tensor())
        outs = _bass_exec_p.bind(
            *operands,
            out_avals=tuple(out_avals),
            in_names=tuple(in_names_full),
            out_names=tuple(out_names),
            lowering_input_output_aliases=(),
            sim_require_finite=True,
            sim_require_nnan=True,
            nc=nc,
        )
        return tuple(outs)

    devices = jax.devices()[:n_cores]
    mesh = Mesh(np.asarray(devices), ("core",))
    in_specs = (PartitionSpec("core"),) * (n_params + len(out_names))
    out_specs = (PartitionSpec("core"),) * len(out_names)
    sharded = jax.jit(
        shard_map(_body, mesh=mesh, in_specs=in_specs, out_specs=out_specs,
                  check_rep=False),
        keep_unused=True)

    def run(in_maps, repeats=1):
        sh = NamedSharding(mesh, PartitionSpec("core"))
        per_core = [[np.asarray(m[k]) for k in in_names] for m in in_maps]
        concat_in = [
            jax.device_put(
                np.concatenate([per_core[c][i] for c in range(n_cores)], axis=0), sh)
            for i in range(n_params)
        ]
        concat_zeros = [
            jax.device_put(
                np.zeros((n_cores * z.shape[0], *z.shape[1:]), z.dtype), sh)
            for z in zero_outs
        ]
        import jax as _j
        _j.block_until_ready(concat_in)
        _j.block_until_ready(concat_zeros)
        times = []
        out_arrs = None
        for _ in range(repeats):
            t0 = time.perf_counter()
            out_arrs = sharded(*concat_in, *concat_zeros)
            _j.block_until_ready(out_arrs)
            times.append(time.perf_counter() - t0)
        results = [
            {name: np.asarray(out_arrs[i]).reshape(n_cores, *out_avals[i].shape)[c]
             for i, name in enumerate(out_names)}
            for c in range(n_cores)
        ]
        return results, times

    return run



def kernel(x, edge_index, iw1, rw1, b1, iw2, rw2, b2, _timing=None, _expose=None):
    x = np.asarray(x, dtype=np.float32)
    edge_index = np.asarray(edge_index)
    in_maps, meta = _preprocess(
        x, edge_index, np.asarray(iw1), np.asarray(rw1), np.asarray(b1),
        np.asarray(iw2), np.asarray(rw2), np.asarray(b2))
    key = ("v8", meta["nchunks"], meta["strm"])
    if key not in _cache:
        nc = _build(meta)
        _cache[key] = (_make_runner(nc), nc)
    run, nc = _cache[key]
    repeats = 30 if _timing is not None else 1
    results, times = run(in_maps, repeats=repeats)
    if _timing is not None:
        _timing.extend(times)
    if _expose is not None:
        _expose.update({"run": run, "in_maps": in_maps, "nc": nc})
    out = np.concatenate([results[c]["out"] for c in range(NC)], axis=0)
    return out



# revision 17
# speedup vs baseline: 2.8844x; 1.0658x over previous
"""ARMA GNN (2-layer, K=2 stacks) distributed Bass kernel for 8 TRN2 NeuronCores.

v2: dst-sorted edge streams + PE selector-matmul segment reduction.
 - Nodes sharded 12500/core; edges partitioned by destination core.
 - Layer math refactored so message passing happens at small feature dims
   (32 cols L1, 16 cols L2), with the stack projections applied before (L1)
   or after (L2) aggregation.
 - The all-gathered per-node table is stored bf16, one node per 256B row.
 - Edge phase per layer: SWDGE dma_gather fetches src rows in dst-sorted
   order (8192 edges/call); for every 128-edge chunk the vector engine
   builds a one-hot selector S^T[j, i] = (dloc[j] == i) and the tensor
   engine accumulates S^T^T @ msg into the dst tile's PSUM bank. No
   scatter descriptors, no conflict packing; f32 PSUM accumulation.
 - agg lives in SBUF; epilogues identical in spirit to v1.
"""
import sys
import time

sys.path.insert(0, "/opt/trn_rl_repo")

import numpy as np
import ml_dtypes

import concourse.bass as bass
import concourse.bacc as bacc
import concourse.mybir as mybir
from concourse.tile import TileContext
from concourse.masks import make_identity
from concourse.library_config import mlp as mlp_lib

BF16 = ml_dtypes.bfloat16

N = 100000
E = 3200000
NC = 8
S = 12500            # nodes per core
NT = 98              # node tiles per core
SP = NT * 128        # 12544 padded nodes per core
QR = 2 * SP          # table rows per quarter (2 core shards)
TBL = NC * SP        # all-gathered table rows
BLK = 1024           # edges per gather call (SWDGE ucode limit)
CHK = 128            # edges per selector chunk
CPC = 16             # gather calls per critical section
NBUF = 32            # rotating msg buffers (>= 2*CPC)
NS = 8               # rotating gather-dma semaphores
FIN, HID, CLS, K = 512, 16, 40, 2

_cache = {}


def _wrap16(idx):
    """[n] int -> [128, n//16] int16: pos i at [i%16, i//16], replicated x8."""
    n = idx.shape[0]
    w = idx.astype(np.int16).reshape(n // 16, 16).T
    return np.ascontiguousarray(np.tile(w, (8, 1)))


def _preprocess(x, edge_index, iw1, rw1, b1, iw2, rw2, b2):
    src = edge_index[0].astype(np.int64)
    dst = edge_index[1].astype(np.int64)
    deg = np.bincount(dst, minlength=N).astype(np.float32)
    dinv = np.where(deg > 0, 1.0 / np.sqrt(deg), 0.0).astype(np.float32)

    core = dst // S
    q = src // (2 * S)                      # source quarter
    trow = (src // S) * SP + (src % S) - QR * q   # quarter-relative table row
    dl = dst - core * S                     # local dst row [0,12500)
    tile = dl // 128
    dloc = dl % 128

    # global (core-uniform) chunk schedule: chunks per (q, tile)
    cnt = np.zeros((NC, 4, NT), np.int64)
    np.add.at(cnt, (core, q, tile), 1)
    kqt = np.ceil(cnt.max(axis=0) / CHK).astype(np.int64)      # [4, NT]
    kqt = np.maximum(kqt, 3)   # >=3 also guards the ds-wait/ceil4 deadlock
    nchunks_q = kqt.sum(axis=1)                                # per quarter
    # call layout per quarter: calls of <=64 chunks
    calls = []                       # list of (q, chunk_off_in_stream, nchunks)
    chunk_base_q = np.zeros(4, np.int64)
    off = 0
    for qq in range(4):
        chunk_base_q[qq] = off
        rem = int(nchunks_q[qq])
        pos = off
        while rem > 0:
            take = min(BLK // CHK, rem)
            calls.append((qq, pos, take))
            pos += take
            rem -= take
        off += int(nchunks_q[qq])
    nchunks = int(off)
    strm = nchunks * CHK

    # chunk start position of each (q, tile)
    tile_chunk_base = np.zeros((4, NT), np.int64)
    for qq in range(4):
        tile_chunk_base[qq] = chunk_base_q[qq] + np.r_[0, np.cumsum(kqt[qq])[:-1]]

    # schedule rows for _build: per chunk -> (q, tile, first, last)
    sched = []
    for qq in range(4):
        for t in range(NT):
            kk = int(kqt[qq, t])
            for j in range(kk):
                sched.append((qq, t, j == 0, j == kk - 1))

    # per-core stream fill
    # ascending src rows inside each (q,tile) group: the SWDGE gather
    # then reads DRAM in ascending order (row-buffer locality); the dloc
    # stream carries the slot mapping so consumption is unaffected.
    order = np.lexsort((trow, tile, q, core))
    src_s = trow[order]
    q_s, t_s, dloc_s = q[order], tile[order], dloc[order]
    core_s = core[order]

    gidx_all, dloc_all = [], []
    for c in range(NC):
        m = core_s == c
        gq, gt, gd, gs = q_s[m], t_s[m], dloc_s[m], src_s[m]
        # rank within (q, tile)
        key = gq * NT + gt
        # edges are sorted by (q, tile, dloc); rank = arange - group start
        starts = np.r_[0, np.flatnonzero(np.diff(key)) + 1]
        grp_start = np.zeros(len(key), np.int64)
        grp_start[starts] = starts
        grp_start = np.maximum.accumulate(grp_start)
        rank = np.arange(len(key)) - grp_start
        pos = tile_chunk_base[gq, gt] * CHK + rank
        gidx = np.zeros(strm, np.int64)
        dlv = np.full(strm, -1.0, np.float32)
        gidx[pos] = gs
        dlv[pos] = gd
        gidx_all.append(_wrap16(gidx))
        dloc_all.append(
            np.ascontiguousarray(
                dlv.reshape(nchunks, CHK).T.astype(BF16)))   # [128, nchunks]

    # weights
    iwcat1 = np.concatenate([iw1[0], iw1[1]], axis=1)        # [512, 32]
    rwcat1 = np.concatenate([rw1[0], rw1[1]], axis=1)        # [512, 32]
    w1 = np.ascontiguousarray(
        np.concatenate([iwcat1, rwcat1], axis=1)).astype(BF16)  # [512, 64]
    b1r = np.tile(np.concatenate([b1[0, 0], b1[1, 0]])[None, :], (128, 1)).astype(np.float32)
    w2 = np.zeros((32, 80), np.float32)
    for k in range(K):
        w2[0:16, 40 * k:40 * k + 40] = iw2[k]
        w2[16:32, 40 * k:40 * k + 40] = rw2[k]
    b2r = np.tile(np.concatenate([b2[0, 0], b2[1, 0]])[None, :], (128, 1)).astype(np.float32)

    iota = np.tile(np.arange(128, dtype=np.float32)[None, :], (128, 1)).astype(BF16)

    in_maps = []
    for c in range(NC):
        xT = np.zeros((FIN, SP), np.float32)
        xT[:, :S] = x[c * S:(c + 1) * S].T
        dv = np.zeros((128, NT), np.float32)
        dvp = np.zeros(SP, np.float32)
        dvp[:S] = dinv[c * S:(c + 1) * S]
        dv[:, :] = dvp.reshape(NT, 128).T
        in_maps.append({
            "xT": np.ascontiguousarray(xT.astype(BF16)),
            "gidx": gidx_all[c],
            "dloc": dloc_all[c],
            "iota": iota,
            "dinv_t": dv,
            "w1": w1,
            "b1r": b1r,
            "w2": w2,
            "b2r": b2r,
        })
    meta = {"calls": calls, "sched": sched, "nchunks": nchunks, "strm": strm}
    return in_maps, meta


def _build(meta):
    calls = meta["calls"]
    sched = meta["sched"]
    nchunks = meta["nchunks"]
    strm = meta["strm"]

    nc = bacc.Bacc("TRN2", target_bir_lowering=False, num_devices=NC,
               num_swdge_queues=4)
    dt = mybir.dt
    f32 = dt.float32
    bf16 = dt.bfloat16

    xT_p = nc.declare_dram_parameter("xT", [FIN, SP], bf16, isOutput=False)
    gidx_p = nc.declare_dram_parameter("gidx", [128, strm // 16], dt.int16, isOutput=False)
    dloc_p = nc.declare_dram_parameter("dloc", [128, nchunks], bf16, isOutput=False)
    iota_p = nc.declare_dram_parameter("iota", [128, 128], bf16, isOutput=False)
    dinv_p = nc.declare_dram_parameter("dinv_t", [128, NT], f32, isOutput=False)
    w1_p = nc.declare_dram_parameter("w1", [FIN, 64], bf16, isOutput=False)
    b1r_p = nc.declare_dram_parameter("b1r", [128, 32], f32, isOutput=False)
    w2_p = nc.declare_dram_parameter("w2", [32, 80], f32, isOutput=False)
    b2r_p = nc.declare_dram_parameter("b2r", [128, 80], f32, isOutput=False)
    out_p = nc.declare_dram_parameter("out", [S, CLS], f32, isOutput=True)

    ag1_in = nc.dram_tensor("ag1_in", [SP, 128], bf16)
    ag1_out = nc.dram_tensor("ag1_out", [TBL, 128], bf16, addr_space="Shared")
    ag2_in = nc.dram_tensor("ag2_in", [SP, 128], bf16)
    ag2_out = nc.dram_tensor("ag2_out", [TBL, 128], bf16, addr_space="Shared")

    gs = [nc.alloc_semaphore(f"gs{i}") for i in range(NS)]
    cc_sem = nc.alloc_semaphore("cc_sem")
    rg = [list(range(NC))]

    with TileContext(nc) as tc:
        with (
            tc.tile_pool(name="const", bufs=1) as cp,
            tc.tile_pool(name="work", bufs=3) as wp,
            tc.tile_pool(name="edge", bufs=1) as ep,
            tc.tile_pool(name="psum", bufs=2, space="PSUM") as pp,
            tc.tile_pool(name="psum1", bufs=2, space="PSUM") as pp1,
            tc.tile_pool(name="selp", bufs=8) as sp,
        ):
            # ---- resident tiles ----
            gidx_sb = cp.tile([128, strm // 16], dt.int16)
            nc.sync.dma_start(gidx_sb[:], gidx_p[:])
            dloc_sb = cp.tile([128, nchunks], bf16)
            nc.sync.dma_start(dloc_sb[:], dloc_p[:])
            iota_sb = cp.tile([128, 128], bf16)
            nc.sync.dma_start(iota_sb[:], iota_p[:])
            dinv_sb = cp.tile([128, NT], f32)
            nc.sync.dma_start(dinv_sb[:], dinv_p[:])
            w1_sb = cp.tile([128, 4, 64], bf16)
            nc.sync.dma_start(w1_sb[:], w1_p[:].rearrange("(k p) n -> p k n", p=128))
            b1r_sb = cp.tile([128, 32], f32)
            nc.sync.dma_start(b1r_sb[:], b1r_p[:])
            w2_sb = cp.tile([32, 80], f32)
            nc.sync.dma_start(w2_sb[:], w2_p[:])
            b2r_sb = cp.tile([128, 80], f32)
            nc.sync.dma_start(b2r_sb[:], b2r_p[:])
            ident = cp.tile([128, 128], f32)
            make_identity(nc, ident[:])
            r1_res = cp.tile([128, NT * 32], f32)
            h_res = cp.tile([128, NT * 16], f32)
            agg1 = cp.tile([128, NT, 32], f32)
            agg2 = cp.tile([128, NT, 16], f32)
            scratch = cp.tile([128, 32], bf16)

            msgs = [ep.tile([128, BLK // CHK, 128], bf16, name=f"msg{i}")
                    for i in range(NBUF)]

            # ---- stage 1: projections x @ [iwcat|rwcat] -> L1 table ----
            for t in range(NT):
                xt = wp.tile([128, 4, 128], bf16, tag="xt")
                nc.sync.dma_start(
                    xt[:], xT_p[:, t * 128:(t + 1) * 128].rearrange("(k p) m -> p k m", p=128))
                hps = pp.tile([128, 64], f32, space="PSUM", tag="hps")
                for k in range(4):
                    nc.tensor.matmul(hps[:], lhsT=xt[:, k, :], rhs=w1_sb[:, k, :],
                                     start=(k == 0), stop=(k == 3))
                h1s = wp.tile([128, 32], bf16, tag="h1s")
                nc.vector.tensor_tensor(
                    out=h1s[:], in0=hps[:, 0:32],
                    in1=dinv_sb[:, t:t + 1].to_broadcast([128, 32]),
                    op=mybir.AluOpType.mult)
                nc.sync.dma_start(ag1_in[t * 128:(t + 1) * 128, 0:32], h1s[:])
                nc.scalar.copy(r1_res[:, 32 * t:32 * t + 32], hps[:, 32:64])

            # ---- stage 2: AllGather L1 table ----
            nc.gpsimd.collective_compute(
                "AllGather", mybir.AluOpType.bypass, replica_groups=rg,
                ins=[ag1_in[:].opt()], outs=[ag1_out[:].opt()])

            # ---- edge phase (shared for both layers) ----
            # chunk ci -> call index
            chunk_call = np.zeros(nchunks, np.int64)
            for j, (qq, coff, nch) in enumerate(calls):
                chunk_call[coff:coff + nch] = j

            # counters persist across the two passes
            cnts = {"g": [0] * NS, "pass": 0}

            def edge_pass(table, agg, fw, epi_cb=None):
                """fw: feature width of rhs slice (32 for L1, 16 for L2).
                epi_cb(t) is emitted right after tile t's final drain so its
                compute overlaps the remaining edge-phase tail."""
                cnts["pass"] += 1
                pn = cnts["pass"]
                ncalls = len(calls)
                gthr = {}

                def finish_group(lo, hi):
                    """Wait for group [lo,hi)'s gather DMAs (keeps desc-gen
                    paced against the ring as in v2)."""
                    for j in range(lo, hi):
                        nc.gpsimd.wait_ge(gs[j % NS], 16 * gthr[j])

                def mark_group(lo, hi):
                    """Re-mark landed buffers (tiny write into the unread
                    junk column zone) so Tile orders consumers after the data
                    actually landed -- on the idle Scalar engine so the
                    887 markers stop serializing the gpsimd queue."""
                    with tc.tile_critical():
                        for j in range(lo, hi):
                            nc.scalar.wait_ge(gs[j % NS], 16 * gthr[j])
                            nc.scalar.memzero(msgs[j % NBUF][0:1, 0:1, 64:66])

                def gather_group(cg):
                    """One critical: issue gathers for calls cg..cg+CPC-1;
                    completion of the PREVIOUS group is waited here, so its
                    DMA tail hides behind this group's descriptor generation."""
                    lo, hi = cg, min(cg + CPC, ncalls)
                    with tc.tile_critical():
                        if cg == 0:
                            # bind collective completion (custom DMA can't
                            # carry walrus waits): probe-read the AG output.
                            nc.gpsimd.memset(scratch[:], 0.0)
                            nc.gpsimd.dma_start(
                                scratch[0:1, 0:32],
                                table[0:1, 0:32]).then_inc(cc_sem, 16)
                            nc.gpsimd.wait_ge(cc_sem, 16 * pn)
                            nc.gpsimd.load_library(mlp_lib)
                        for j in range(lo, hi):
                            qq, coff, nch = calls[j]
                            eoff = coff * CHK
                            nidx = nch * CHK
                            cnts["g"][j % NS] += 1
                            gthr[j] = cnts["g"][j % NS]
                            nc.gpsimd.dma_gather(
                                out_ap=msgs[j % NBUF][:, 0:nch, :],
                                in_ap=table[QR * qq:QR * (qq + 1), :],
                                idxs_ap=gidx_sb[:, eoff // 16:(eoff + nidx) // 16],
                                num_idxs=nidx, num_idxs_reg=nidx, elem_size=128,
                                queue_num=j % 4,
                            ).then_inc(gs[j % NS], 16)
                        if cg > 0:
                            finish_group(cg - CPC, cg)
                    if cg > 0:
                        mark_group(cg - CPC, cg)

                def tail_group(cg):
                    with tc.tile_critical():
                        finish_group(cg, ncalls)
                    mark_group(cg, ncalls)

                # Tile-managed consumption; gather criticals interleave so
                # the scheduler can overlap desc-gen with PE/DVE consumption.
                issued = 0
                pst = None
                last_lo = ((ncalls - 1) // CPC) * CPC
                tail_done = False
                for ci, (qq, t, first, last) in enumerate(sched):
                    j = int(chunk_call[ci])
                    # issue gather groups ahead of consumption; the bound
                    # guarantees no buffer in the group can overwrite data
                    # whose reader instructions aren't emitted yet.
                    while issued < ncalls and issued <= j + NBUF - CPC:
                        gather_group(issued)
                        issued += CPC
                    if j >= last_lo and not tail_done:
                        tail_done = True
                        tail_group(last_lo)
                    sel = sp.tile([128, 128], bf16, tag="sel")
                    nc.vector.tensor_tensor(
                        out=sel[:], in0=iota_sb[:],
                        in1=dloc_sb[:, ci:ci + 1].to_broadcast([128, 128]),
                        op=mybir.AluOpType.is_equal)
                    if first:
                        pst = pp1.tile([128, fw], f32, space="PSUM", tag="eps")
                    cin = ci - calls[j][1]
                    nc.tensor.matmul(
                        pst[:], lhsT=sel[:],
                        rhs=msgs[j % NBUF][:, cin, 0:fw],
                        start=first, stop=last)
                    if last:
                        sl = agg[:, t, :]
                        if qq == 0:
                            nc.vector.tensor_scalar(
                                out=sl, in0=pst[:], scalar1=1.0,
                                scalar2=None, op0=mybir.AluOpType.mult)
                        else:
                            nc.vector.tensor_tensor(
                                out=sl, in0=pst[:], in1=sl,
                                op=mybir.AluOpType.add)
                        if qq == 3 and epi_cb is not None:
                            epi_cb(t)

            def epi1(t):
                dvb = dinv_sb[:, t:t + 1].to_broadcast([128, 32])
                v = wp.tile([128, 32], f32, tag="v")
                nc.vector.tensor_tensor(out=v[:], in0=agg1[:, t, :], in1=dvb,
                                        op=mybir.AluOpType.mult)
                nc.vector.tensor_tensor(out=v[:], in0=v[:], in1=r1_res[:, 32 * t:32 * t + 32],
                                        op=mybir.AluOpType.add)
                nc.vector.tensor_tensor(out=v[:], in0=v[:], in1=b1r_sb[:],
                                        op=mybir.AluOpType.add)
                nc.vector.tensor_scalar(out=v[:], in0=v[:], scalar1=0.0, scalar2=None,
                                        op0=mybir.AluOpType.max)
                h = h_res[:, 16 * t:16 * t + 16]
                nc.vector.tensor_tensor(out=h, in0=v[:, 0:16], in1=v[:, 16:32],
                                        op=mybir.AluOpType.add)
                nc.vector.tensor_scalar(out=h, in0=h, scalar1=0.5, scalar2=None,
                                        op0=mybir.AluOpType.mult)
                h2s = wp.tile([128, 16], bf16, tag="h2s")
                nc.vector.tensor_tensor(
                    out=h2s[:], in0=h,
                    in1=dinv_sb[:, t:t + 1].to_broadcast([128, 16]),
                    op=mybir.AluOpType.mult)
                nc.sync.dma_start(ag2_in[t * 128:(t + 1) * 128, 0:16], h2s[:])

            edge_pass(ag1_out, agg1, 32, epi_cb=epi1)

            # ---- AllGather L2 table ----
            nc.gpsimd.collective_compute(
                "AllGather", mybir.AluOpType.bypass, replica_groups=rg,
                ins=[ag2_in[:].opt()], outs=[ag2_out[:].opt()])

            # ---- L2 epilogue (per tile, overlapped via epi_cb) ----
            def epi2(t):
                cc = wp.tile([128, 32], f32, tag="cc")
                nc.vector.tensor_tensor(
                    out=cc[:, 0:16], in0=agg2[:, t, :],
                    in1=dinv_sb[:, t:t + 1].to_broadcast([128, 16]),
                    op=mybir.AluOpType.mult)
                nc.scalar.copy(cc[:, 16:32], h_res[:, 16 * t:16 * t + 16])
                ccT_ps = pp.tile([32, 128], f32, space="PSUM", tag="ccT")
                nc.tensor.transpose(out=ccT_ps[:], in_=cc[:], identity=ident[:])
                ccT = wp.tile([32, 128], f32, tag="ccTs")
                nc.scalar.copy(ccT[:], ccT_ps[:])
                ops = pp.tile([128, 80], f32, space="PSUM", tag="ops")
                nc.tensor.matmul(ops[:], lhsT=ccT[:], rhs=w2_sb[:], start=True, stop=True)
                o = wp.tile([128, 80], f32, tag="o")
                nc.vector.tensor_tensor(out=o[:], in0=ops[:], in1=b2r_sb[:],
                                        op=mybir.AluOpType.add)
                nc.vector.tensor_scalar(out=o[:], in0=o[:], scalar1=0.0, scalar2=None,
                                        op0=mybir.AluOpType.max)
                fin = wp.tile([128, CLS], f32, tag="fin")
                nc.vector.tensor_tensor(out=fin[:], in0=o[:, 0:40], in1=o[:, 40:80],
                                        op=mybir.AluOpType.add)
                nc.vector.tensor_scalar(out=fin[:], in0=fin[:], scalar1=0.5, scalar2=None,
                                        op0=mybir.AluOpType.mult)
                lo = t * 128
                hi = min(lo + 128, S)
                if hi > lo:
                    nc.sync.dma_start(out_p[lo:hi, :], fin[0:hi - lo, :])

            edge_pass(ag2_out, agg2, 16, epi_cb=epi2)

    nc.compile()
    return nc


def _make_runner(nc, n_cores=NC):
    import jax
    from jax.sharding import Mesh, PartitionSpec, NamedSharding
    from jax.experimental.shard_map import shard_map
    from concourse.bass2jax import (
        _bass_exec_p, install_neuronx_cc_hook, partition_id_tensor)

    install_neuronx_cc_hook()
    partition_name = nc.partition_id_tensor.name if nc.partition_id_tensor else None
    in_names, out_names, out_avals, zero_outs = [], [], [], []
    for alloc in nc.m.functions[0].allocations:
        if not isinstance(alloc, mybir.MemoryLocationSet):
            continue
        name = alloc.memorylocations[0].name
        if alloc.kind == "ExternalInput":
            if name != partition_name:
                in_names.append(name)
        elif alloc.kind == "ExternalOutput":
            out_names.append(name)
            shape = tuple(alloc.tensor_shape)
            dtype = mybir.dt.np(alloc.dtype)
            out_avals.append(jax.core.ShapedArray(shape, dtype))
            zero_outs.append(np.zeros(shape, dtype))
    n_params = len(in_names)
    in_names_full = list(in_names) + out_names
    if partition_name is not None:
        in_names_full.append(partition_name)

    def _body(*args):
        operands = list(args)
        if partition_name is not None:
            operands.append(partition_id_tensor())
        outs = _bass_exec_p.bind(
            *operands,
            out_avals=tuple(out_avals),
            in_names=tuple(in_names_full),
            out_names=tuple(out_names),
            lowering_input_output_aliases=(),
            sim_require_finite=True,
            sim_require_nnan=True,
            nc=nc,
        )
        return tuple(outs)

    devices = jax.devices()[:n_cores]
    mesh = Mesh(np.asarray(devices), ("core",))
    in_specs = (PartitionSpec("core"),) * (n_params + len(out_names))
    out_specs = (PartitionSpec("core"),) * len(out_names)
    sharded = jax.jit(
        shard_map(_body, mesh=mesh, in_specs=in_specs, out_specs=out_specs,
                  check_rep=False),
        keep_unused=True)

    def run(in_maps, repeats=1):
        sh = NamedSharding(mesh, PartitionSpec("core"))
        per_core = [[np.asarray(m[k]) for k in in_names] for m in in_maps]
        concat_in = [
            jax.device_put(
                np.concatenate([per_core[c][i] for c in range(n_cores)], axis=0), sh)
            for i in range(n_params)
        ]
        concat_zeros = [
            jax.device_put(
                np.zeros((n_cores * z.shape[0], *z.shape[1:]), z.dtype), sh)
            for z in zero_outs
        ]
        import jax as _j
        _j.block_until_ready(concat_in)
        _j.block_until_ready(concat_zeros)
        times = []
        out_arrs = None
        for _ in range(repeats):
            t0 = time.perf_counter()
            out_arrs = sharded(*concat_in, *concat_zeros)
            _j.block_until_ready(out_arrs)
            times.append(time.perf_counter() - t0)
        results = [
            {name: np.asarray(out_arrs[i]).reshape(n_cores, *out_avals[i].shape)[c]
             for i, name in enumerate(out_names)}
            for c in range(n_cores)
        ]
        return results, times

    return run



def kernel(x, edge_index, iw1, rw1, b1, iw2, rw2, b2, _timing=None, _expose=None):
    x = np.asarray(x, dtype=np.float32)
    edge_index = np.asarray(edge_index)
    in_maps, meta = _preprocess(
        x, edge_index, np.asarray(iw1), np.asarray(rw1), np.asarray(b1),
        np.asarray(iw2), np.asarray(rw2), np.asarray(b2))
    key = ("v9", meta["nchunks"], meta["strm"])
    if key not in _cache:
        nc = _build(meta)
        _cache[key] = (_make_runner(nc), nc)
    run, nc = _cache[key]
    repeats = 30 if _timing is not None else 1
    results, times = run(in_maps, repeats=repeats)
    if _timing is not None:
        _timing.extend(times)
    if _expose is not None:
        _expose.update({"run": run, "in_maps": in_maps, "nc": nc})
    out = np.concatenate([results[c]["out"] for c in range(NC)], axis=0)
    return out

